# revision 1
# baseline (speedup 1.0000x reference)
"""CoAttention ImageDNS kernel for Trainium2 (8 NeuronCores, Bass/Tile).

Math: the reference computes two additive-attention blocks. In both, the
softmax'd score is  score[b, q, k] = f(q-side)[b, q] + g(k-side)[b, k] + c,
and softmax over k is invariant to the q-dependent (and constant) terms, so
the attention weights are independent of the query index:

  visual_att[b, s, :]  = softmax_r( wB . tanh(W_i1 @ img[b, r]) )
  textual_att[b, i, :] = softmax_j( wD . tanh(W_d2 @ dns[b, j]) )

Hence both outputs are per-batch rank-1 broadcasts:

  att_img_features[b, s, :] = visual_att[b]  @ img[b]   (same for all s)
  att_dns_features[b, i, :] = textual_att[b] @ dns[b]   (same for all i)

W_d1/b_d1/w_att1[:H]/b_att1/W_i2/b_i2/w_att2[:H]/b_att2 cancel entirely.

Sharding: pure data-parallel over batch, 4 batches per core, no collectives.
Matmul operands are bf16 (fp16 streams at half rate on the trn2 PE; bf16
end-to-end rel err ~3e-3 vs the fp32 reference); accumulation is fp32 in
PSUM, softmax/normalization in fp32.
"""

import sys
import numpy as np
import ml_dtypes

_BF16 = ml_dtypes.bfloat16

for _p in ("/opt/trn_rl_repo", "/root/.axon_site/_ro/trn_rl_repo"):
    if _p not in sys.path:
        sys.path.append(_p)

B, S, R, H = 32, 512, 196, 1024
NCORES = 8
BLOC = B // NCORES          # batches per core
OC = 512                    # output-chunk (one fp32 PSUM bank)
HC = H // 128               # contraction chunks

_CACHE = {}


def _row_chunks(n):
    out, o = [], 0
    while o < n:
        out.append((o, min(128, n - o)))
        o += 128
    return out


def build_nc():
    from concourse import bacc, mybir
    from concourse import tile

    f32, f16 = mybir.dt.float32, mybir.dt.bfloat16
    Act = mybir.ActivationFunctionType
    Alu = mybir.AluOpType

    nc = bacc.Bacc("TRN2", target_bir_lowering=False, debug=False)

    RP = 256  # img row count padded to a partition multiple for single-DMA loads
    xt_dns = nc.dram_tensor("xt_dns", [BLOC, HC, 128, S], f16, kind="ExternalInput")
    xn_dns = nc.dram_tensor("xn_dns", [BLOC, S, H], f16, kind="ExternalInput")
    xt_img = nc.dram_tensor("xt_img", [BLOC, HC, 128, R], f16, kind="ExternalInput")
    xn_img = nc.dram_tensor("xn_img", [BLOC, RP, H], f16, kind="ExternalInput")
    wt_i1 = nc.dram_tensor("wt_i1", [HC, 128, H], f16, kind="ExternalInput")
    wt_d2 = nc.dram_tensor("wt_d2", [HC, 128, H], f16, kind="ExternalInput")
    wrow_b = nc.dram_tensor("wrow_b", [128, H], f32, kind="ExternalInput")
    wrow_d = nc.dram_tensor("wrow_d", [128, H], f32, kind="ExternalInput")
    out_dns = nc.dram_tensor("out_dns", [BLOC, S, H], f32, kind="ExternalOutput")
    out_img = nc.dram_tensor("out_img", [BLOC, S, H], f32, kind="ExternalOutput")

    with tile.TileContext(nc) as tc:
        with (
            tc.tile_pool(name="const", bufs=1) as cpool,
            tc.tile_pool(name="xts", bufs=2) as xtpool,
            tc.tile_pool(name="xns", bufs=2) as xnpool,
            tc.tile_pool(name="work", bufs=3) as wpool,
            tc.tile_pool(name="small", bufs=12) as spool,
            tc.tile_pool(name="outs", bufs=2) as opool,
            tc.tile_pool(name="pp", bufs=3, space="PSUM") as ppool,
            tc.tile_pool(name="ps", bufs=2, space="PSUM") as pstat,
        ):
            # lazy const loads: weight DMAs are interleaved with the first
            # activation loads (per-hc) at first use, so the first projection
            # group's dependencies land early in the queue
            wt_sb, wrow_sb = {}, {}

            def get_wt(nm):
                if nm not in wt_sb:
                    w = cpool.tile([128, HC * H], f16, name=f"wt_{nm}_sb")
                    wt_sb[nm] = w
                return wt_sb[nm]

            def load_wt_chunk(nm, hc):
                dram = {"i1": wt_i1, "d2": wt_d2}[nm]
                w = wt_sb[nm]
                nc.sync.dma_start(out=w[:, hc * H:(hc + 1) * H], in_=dram[hc])

            def get_wrow(nm):
                if nm not in wrow_sb:
                    dram = {"b": wrow_b, "d": wrow_d}[nm]
                    w = cpool.tile([128, H], f32, name=f"wrow_{nm}_sb")
                    nc.sync.dma_start(out=w[:, :], in_=dram[:, :])
                    wrow_sb[nm] = w
                return wrow_sb[nm]

            ones_col = cpool.tile([128, 1], f16, name="ones_col")
            nc.vector.memset(ones_col[:, :], 1.0)
            ones_row = cpool.tile([1, 128], f32, name="ones_row")
            nc.vector.memset(ones_row[:, :], 1.0)

            for b in range(BLOC):
                for side in ("img", "dns"):
                    n_rows = R if side == "img" else S
                    xt_d = xt_img if side == "img" else xt_dns
                    xn_d = xn_img if side == "img" else xn_dns
                    wt_name = "i1" if side == "img" else "d2"
                    load_wt = wt_name not in wt_sb
                    wt = get_wt(wt_name)
                    out_d = out_img if side == "img" else out_dns
                    rcs = _row_chunks(n_rows)

                    # -- loads: on a weight's first use, interleave per-hc wt/xt
                    # chunks so the first projection group's deps land first;
                    # afterwards one 3D DMA covers the whole xt tile --
                    xt_t = xtpool.tile([128, HC * n_rows], f16,
                                       name=f"xt_{side}_{b}", tag=f"xt_{side}")
                    if load_wt:
                        for hc in range(HC):
                            load_wt_chunk(wt_name, hc)
                            nc.sync.dma_start(
                                out=xt_t[:, hc * n_rows:(hc + 1) * n_rows],
                                in_=xt_d[b, hc])
                    else:
                        nc.sync.dma_start(
                            out=xt_t.rearrange("p (hc m) -> p hc m", hc=HC),
                            in_=xt_d[b].rearrange("hc p m -> p hc m"))

                    # -- projection, tanh, weighted o-reduction, exp --
                    # xn / wrow loads are issued after the first proj group so
                    # the projection's own dependencies lead the DMA queues
                    acols = []
                    xn_ts = []
                    wr = None
                    s_ps = pstat.tile([1, 1], f32, name=f"s_{side}_{b}", tag="stat")
                    for ci, (r0, rk) in enumerate(rcs):
                        ps = ppool.tile([128, H], f32, name=f"proj_{side}_{ci}_{b}",
                                        tag="pp")
                        for hc in range(HC):
                            lhs = xt_t[:, hc * n_rows + r0: hc * n_rows + r0 + rk]
                            for oc in range(2):
                                nc.tensor.matmul(
                                    ps[0:rk, oc * OC:(oc + 1) * OC],
                                    lhsT=lhs,
                                    rhs=wt[:, hc * H + oc * OC: hc * H + (oc + 1) * OC],
                                    start=(hc == 0), stop=(hc == HC - 1))
                        if ci == 0:
                            nrc = len(rcs)
                            xn_t = xnpool.tile([128, nrc * H], f16,
                                               name=f"xn_{side}_{b}", tag=f"xn_{side}")
                            nc.sync.dma_start(
                                out=xn_t.rearrange("p (rc n) -> p rc n", rc=nrc),
                                in_=xn_d[b, 0:nrc * 128, :]
                                .rearrange("(rc p) n -> p rc n", p=128))
                            xn_ts = [xn_t[:, cj * H:(cj + 1) * H] for cj in range(nrc)]
                            wr = get_wrow("b" if side == "img" else "d")
                        th = wpool.tile([128, H], f32, name=f"th_{side}_{ci}_{b}", tag="th")
                        nc.scalar.activation(th[0:rk, :], ps[0:rk, :], Act.Tanh)
                        scr = wpool.tile([128, H], f32, name=f"scr_{side}_{ci}_{b}",
                                         tag="scr", bufs=2)
                        tcol = spool.tile([128, 1], f32, name=f"tc_{side}_{ci}_{b}", tag="tcol")
                        nc.vector.scalar_tensor_tensor(
                            out=scr[0:rk, :], in0=th[0:rk, :], scalar=1.0,
                            in1=wr[0:rk, :], op0=Alu.mult, op1=Alu.mult,
                            accum_out=tcol[0:rk, :])
                        acol = spool.tile([128, 1], f16, name=f"a_{side}_{ci}_{b}",
                                          tag=f"acol_{side}_{ci}")
                        nc.scalar.activation(acol[0:rk, :], tcol[0:rk, :], Act.Exp)
                        acols.append((acol, rk))
                        nc.tensor.matmul(
                            s_ps[0:1, 0:1], lhsT=acol[0:rk, 0:1], rhs=ones_col[0:rk, 0:1],
                            start=(ci == 0), stop=(ci == len(rcs) - 1))

                    # -- 1/sum, broadcast to 128 partitions (idle GPSIMD) --
                    r_sb = spool.tile([1, 1], f32, name=f"r_{side}_{b}", tag="r")
                    nc.vector.reciprocal(r_sb[0:1, 0:1], s_ps[0:1, 0:1])
                    rb_sb = spool.tile([128, 1], f32, name=f"rbs_{side}_{b}", tag="rb")
                    nc.gpsimd.partition_broadcast(rb_sb[:, 0:1], r_sb[0:1, 0:1])

                    # -- stage 2: out[s, h] = sum_r a_r x[r, h], all 128 s at once --
                    att_ps = ppool.tile([128, H], f32, name=f"att_{side}_{b}", tag="pp")
                    for h2 in range(2):
                        for ci, (r0, rk) in enumerate(rcs):
                            acol, _ = acols[ci]
                            nc.tensor.matmul(
                                att_ps[:, h2 * OC:(h2 + 1) * OC],
                                lhsT=acol[0:rk, 0:1].to_broadcast((rk, 128)),
                                rhs=xn_ts[ci][0:rk, h2 * OC:(h2 + 1) * OC],
                                start=(ci == 0), stop=(ci == len(rcs) - 1))
                    att_sb = opool.tile([128, H], f32, name=f"attsb_{side}_{b}",
                                        tag=f"att_{side}")
                    for h2 in range(2):
                        nc.scalar.activation(att_sb[:, h2 * OC:(h2 + 1) * OC],
                                             att_ps[:, h2 * OC:(h2 + 1) * OC],
                                             Act.Copy, scale=rb_sb[:, 0:1])
                        # broadcast DMA: all 512 output rows of this h-half
                        nc.sync.dma_start(
                            out=out_d[b, :, h2 * OC:(h2 + 1) * OC]
                            .rearrange("(sc p) n -> p sc n", p=128),
                            in_=att_sb[:, h2 * OC:(h2 + 1) * OC]
                            .rearrange("p (o n) -> p o n", o=1)
                            .to_broadcast((128, S // 128, OC)))
    nc.compile()
    return nc


def _get_nc():
    if "nc" not in _CACHE:
        _CACHE["nc"] = build_nc()
    return _CACHE["nc"]


def make_in_maps(inputs):
    dns = np.ascontiguousarray(np.asarray(inputs["dns_feature"], dtype=np.float32))
    img = np.ascontiguousarray(np.asarray(inputs["img_features"], dtype=np.float32))
    W_i1 = np.asarray(inputs["W_i1"], dtype=np.float32)
    W_d2 = np.asarray(inputs["W_d2"], dtype=np.float32)
    wB = np.asarray(inputs["w_att1"], dtype=np.float32)[H:]
    wD = np.asarray(inputs["w_att2"], dtype=np.float32)[H:]

    wt_i1 = np.ascontiguousarray(W_i1.T).reshape(HC, 128, H).astype(_BF16)
    wt_d2 = np.ascontiguousarray(W_d2.T).reshape(HC, 128, H).astype(_BF16)
    wrow_b = np.ascontiguousarray(np.broadcast_to(wB, (128, H)))
    wrow_d = np.ascontiguousarray(np.broadcast_to(wD, (128, H)))

    xt_dns = np.ascontiguousarray(
        dns.transpose(0, 2, 1).reshape(B, HC, 128, S).astype(_BF16))
    xt_img = np.ascontiguousarray(
        img.transpose(0, 2, 1).reshape(B, HC, 128, R).astype(_BF16))
    xn_dns = dns.astype(_BF16)
    xn_img = np.zeros((B, 256, H), dtype=_BF16)
    xn_img[:, :R, :] = img.astype(_BF16)

    in_maps = []
    for k in range(NCORES):
        sl = slice(k * BLOC, (k + 1) * BLOC)
        in_maps.append({
            "xt_dns": np.ascontiguousarray(xt_dns[sl]),
            "xn_dns": np.ascontiguousarray(xn_dns[sl]),
            "xt_img": np.ascontiguousarray(xt_img[sl]),
            "xn_img": np.ascontiguousarray(xn_img[sl]),
            "wt_i1": wt_i1,
            "wt_d2": wt_d2,
            "wrow_b": wrow_b,
            "wrow_d": wrow_d,
        })
    return in_maps


def kernel(**inputs):
    from concourse.bass_utils import run_bass_kernel_spmd

    nc = _get_nc()
    in_maps = make_in_maps(inputs)
    res = run_bass_kernel_spmd(nc, in_maps, list(range(NCORES))).results
    att_dns = np.concatenate([res[k]["out_dns"] for k in range(NCORES)], axis=0)
    att_img = np.concatenate([res[k]["out_img"] for k in range(NCORES)], axis=0)
    return att_dns, att_img



# revision 3
# speedup vs baseline: 1.1223x; 1.1223x over previous
"""CoAttention ImageDNS kernel for Trainium2 (8 NeuronCores, Bass/Tile).

Math: the reference computes two additive-attention blocks. In both, the
softmax'd score is  score[b, q, k] = f(q-side)[b, q] + g(k-side)[b, k] + c,
and softmax over k is invariant to the q-dependent (and constant) terms, so
the attention weights are independent of the query index:

  visual_att[b, s, :]  = softmax_r( wB . tanh(W_i1 @ img[b, r]) )
  textual_att[b, i, :] = softmax_j( wD . tanh(W_d2 @ dns[b, j]) )

Hence both outputs are per-batch rank-1 broadcasts:

  att_img_features[b, s, :] = visual_att[b]  @ img[b]   (same for all s)
  att_dns_features[b, i, :] = textual_att[b] @ dns[b]   (same for all i)

W_d1/b_d1/w_att1[:H]/b_att1/W_i2/b_i2/w_att2[:H]/b_att2 cancel entirely.

Sharding: pure data-parallel over batch, 4 batches per core, no collectives.

Perf notes vs the bf16 baseline (140.2us):
- Projections run as fp8(e4m3) DoubleRow matmuls (2x PE throughput). To keep
  accuracy, the DoubleRow pair slots carry a Kahan-style weight split:
  pair = (W_hi, x) + (W_lo, x) with W_hi = e4m3(64*W), W_lo = e4m3(64*W-W_hi),
  so the weight operand is ~bf16-exact and only x carries e4m3 rounding
  (end-to-end rel err ~1.7e-2 vs the 2e-2 gate; the 'B' scheme also splits x
  and lands at ~1.8e-3 at 1.5x the matmul cost — switch SCHEME if needed).
- The tiny score-sum and stage-2 matmuls for item k are emitted AFTER all of
  item k+1's projection matmuls, so the PE stream never waits on the
  scalar/vector tanh/score chain (PE idle gaps re-throttle the HAM clock
  gate from 2.4GHz to 1.2GHz, which is what capped the baseline).
- Only one [1, H] output row per (batch, side) leaves the device; the
  broadcast over S is done on host (kills 16MB/core of output DMA).
- Stage 2 (attention-weighted sum of rows) stays bf16: e4m3 there would put
  ~3.6% error directly on the output.
"""

import sys
import numpy as np
import ml_dtypes

_BF16 = ml_dtypes.bfloat16
_E4M3 = ml_dtypes.float8_e4m3

for _p in ("/opt/trn_rl_repo", "/root/.axon_site/_ro/trn_rl_repo"):
    if _p not in sys.path:
        sys.path.append(_p)

B, S, R, H = 32, 512, 196, 1024
NCORES = 8
BLOC = B // NCORES          # batches per core
OC = 512                    # output-chunk (one fp32 PSUM bank)
HC = H // 128               # 128-row contraction blocks
WSCALE = 64.0               # W quantization pre-scale (undone in tanh's scale)

# 'D': W split hi+lo, x plain e4m3  (fastest, rel err ~1.7e-2)
# 'H': D + x residual correction on blocks 0..3 (~1.2e-2, +25% matmul cost)
# 'B': D + x residual correction on all blocks  (~1.8e-3, +50% matmul cost)
SCHEME = "D"

_CACHE = {}


def _row_chunks(n):
    out, o = [], 0
    while o < n:
        out.append((o, min(128, n - o)))
        o += 128
    return out


def build_nc():
    from concourse import bacc, mybir
    from concourse import tile

    f32, f16, f8 = mybir.dt.float32, mybir.dt.bfloat16, mybir.dt.float8e4
    Act = mybir.ActivationFunctionType
    Alu = mybir.AluOpType
    DR = mybir.MatmulPerfMode.DoubleRow

    nc = bacc.Bacc("TRN2", target_bir_lowering=False, debug=False)

    n_xlo = {"D": 0, "H": 2, "B": 4}[SCHEME]  # x-residual block-pairs per oc

    RP = 256  # img row count padded to a partition multiple for single-DMA loads
    xt_dns = nc.dram_tensor("xt_dns", [BLOC, HC, 128, S], f8, kind="ExternalInput")
    xt_img = nc.dram_tensor("xt_img", [BLOC, HC, 128, R], f8, kind="ExternalInput")
    xn_dns = nc.dram_tensor("xn_dns", [BLOC, S, H], f16, kind="ExternalInput")
    xn_img = nc.dram_tensor("xn_img", [BLOC, RP, H], f16, kind="ExternalInput")
    wt_i1 = nc.dram_tensor("wt_i1", [HC, 128, 2, H], f8, kind="ExternalInput")
    wt_d2 = nc.dram_tensor("wt_d2", [HC, 128, 2, H], f8, kind="ExternalInput")
    wrow_b = nc.dram_tensor("wrow_b", [128, H], f32, kind="ExternalInput")
    wrow_d = nc.dram_tensor("wrow_d", [128, H], f32, kind="ExternalInput")
    if n_xlo:
        xl_dns = nc.dram_tensor("xl_dns", [BLOC, HC, 128, S], f8, kind="ExternalInput")
        xl_img = nc.dram_tensor("xl_img", [BLOC, HC, 128, R], f8, kind="ExternalInput")
    out_rows = nc.dram_tensor("out_rows", [BLOC, 2, H], f32, kind="ExternalOutput")

    with tile.TileContext(nc) as tc:
        with (
            tc.tile_pool(name="const", bufs=1) as cpool,
            tc.tile_pool(name="xts", bufs=2) as xtpool,
            tc.tile_pool(name="xns", bufs=2) as xnpool,
            tc.tile_pool(name="work", bufs=3) as wpool,
            tc.tile_pool(name="small", bufs=2) as spool,
            tc.tile_pool(name="outs", bufs=2) as opool,
            tc.tile_pool(name="pp", bufs=2, space="PSUM") as ppool,
            tc.tile_pool(name="pa", bufs=2, space="PSUM") as papool,
            tc.tile_pool(name="ps", bufs=2, space="PSUM") as pstat,
        ):
            wt_sb, wrow_sb = {}, {}

            def get_wrow(nm):
                if nm not in wrow_sb:
                    dram = {"b": wrow_b, "d": wrow_d}[nm]
                    w = cpool.tile([128, H], f32, name=f"wrow_{nm}_sb")
                    nc.sync.dma_start(out=w[:, :], in_=dram[:, :])
                    wrow_sb[nm] = w
                return wrow_sb[nm]

            ones_col = cpool.tile([128, 1], f16, name="ones_col")
            nc.vector.memset(ones_col[:, :], 1.0)

            # state of a finished projection phase, consumed one item later
            def emit_proj(b, side):
                n_rows = R if side == "img" else S
                xt_d = xt_img if side == "img" else xt_dns
                xn_d = xn_img if side == "img" else xn_dns
                wt_name = "i1" if side == "img" else "d2"
                load_wt = wt_name not in wt_sb
                if load_wt:
                    w = cpool.tile([128, HC * 2 * H], f8, name=f"wt_{wt_name}_sb")
                    wt_sb[wt_name] = w
                wt = wt_sb[wt_name]
                # [128, hc, var, o] view of the weight tile
                wv = wt.rearrange("p (hc v o) -> p hc v o", hc=HC, v=2)
                rcs = _row_chunks(n_rows)

                # -- loads: on a weight's first use, interleave per-hc wt/xt
                # chunks so the first projection group's deps land first --
                xt_t = xtpool.tile([128, HC * n_rows], f8,
                                   name=f"xt_{side}_{b}", tag=f"xt_{side}")
                wt_dram = {"i1": wt_i1, "d2": wt_d2}[wt_name]
                if load_wt:
                    for hc in range(HC):
                        nc.sync.dma_start(
                            out=wv[:, hc], in_=wt_dram[hc])
                        nc.sync.dma_start(
                            out=xt_t[:, hc * n_rows:(hc + 1) * n_rows],
                            in_=xt_d[b, hc])
                else:
                    nc.sync.dma_start(
                        out=xt_t.rearrange("p (hc m) -> p hc m", hc=HC),
                        in_=xt_d[b].rearrange("hc p m -> p hc m"))
                xv = xt_t.rearrange("p (hc m) -> p hc m", hc=HC)
                if n_xlo:
                    xl_d = xl_img if side == "img" else xl_dns
                    xl_t = xtpool.tile([128, HC * n_rows], f8,
                                       name=f"xl_{side}_{b}", tag=f"xl_{side}")
                    nc.sync.dma_start(
                        out=xl_t.rearrange("p (hc m) -> p hc m", hc=HC),
                        in_=xl_d[b].rearrange("hc p m -> p hc m"))
                    xlv = xl_t.rearrange("p (hc m) -> p hc m", hc=HC)

                # -- projection (fp8 DoubleRow), tanh, weighted o-reduce, exp --
                acols = []
                xn_ts = []
                wr = None
                for ci, (r0, rk) in enumerate(rcs):
                    ps = ppool.tile([128, H], f32, name=f"proj_{side}_{ci}_{b}",
                                    tag="pp")
                    for hc in range(HC):
                        lhs = xv[:, hc:hc + 1, r0:r0 + rk].to_broadcast(
                            (128, 2, rk))
                        for oc in range(2):
                            nc.tensor.matmul(
                                ps[0:rk, oc * OC:(oc + 1) * OC],
                                lhsT=lhs,
                                rhs=wv[:, hc, :, oc * OC:(oc + 1) * OC],
                                start=(hc == 0), stop=(hc == HC - 1 and not n_xlo),
                                perf_mode=DR)
                    for c in range(n_xlo):
                        lhs = xlv[:, 2 * c:2 * c + 2, r0:r0 + rk]
                        for oc in range(2):
                            nc.tensor.matmul(
                                ps[0:rk, oc * OC:(oc + 1) * OC],
                                lhsT=lhs,
                                rhs=wv[:, 2 * c:2 * c + 2, 0, oc * OC:(oc + 1) * OC],
                                start=False, stop=(c == n_xlo - 1),
                                perf_mode=DR)
                    if ci == 0:
                        nrc = len(rcs)
                        xn_t = xnpool.tile([128, nrc * H], f16,
                                           name=f"xn_{side}_{b}", tag=f"xn_{side}")
                        nc.sync.dma_start(
                            out=xn_t.rearrange("p (rc n) -> p rc n", rc=nrc),
                            in_=xn_d[b, 0:nrc * 128, :]
                            .rearrange("(rc p) n -> p rc n", p=128))
                        xn_ts = [xn_t[:, cj * H:(cj + 1) * H] for cj in range(nrc)]
                        wr = get_wrow("b" if side == "img" else "d")
                    th = wpool.tile([128, H], f32, name=f"th_{side}_{ci}_{b}", tag="th")
                    nc.scalar.activation(th[0:rk, :], ps[0:rk, :], Act.Tanh,
                                         scale=1.0 / WSCALE)
                    scr = wpool.tile([128, H], f32, name=f"scr_{side}_{ci}_{b}",
                                     tag="scr", bufs=2)
                    tcol = spool.tile([128, 1], f32, name=f"tc_{side}_{ci}_{b}",
                                      tag="tcol", bufs=3)
                    nc.vector.scalar_tensor_tensor(
                        out=scr[0:rk, :], in0=th[0:rk, :], scalar=1.0,
                        in1=wr[0:rk, :], op0=Alu.mult, op1=Alu.mult,
                        accum_out=tcol[0:rk, :])
                    acol = spool.tile([128, 1], f16, name=f"a_{side}_{ci}_{b}",
                                      tag=f"acol_{side}_{ci}", bufs=2)
                    nc.scalar.activation(acol[0:rk, :], tcol[0:rk, :], Act.Exp)
                    acols.append((acol, rk))
                return (b, side, acols, xn_ts)

            # sum-of-exps, reciprocal, stage-2 weighted sum, output row DMA
            def emit_reduce(state):
                b, side, acols, xn_ts = state
                sd = 0 if side == "img" else 1
                s_ps = pstat.tile([1, 1], f32, name=f"s_{side}_{b}", tag="stat")
                for ci, (acol, rk) in enumerate(acols):
                    nc.tensor.matmul(
                        s_ps[0:1, 0:1], lhsT=acol[0:rk, 0:1],
                        rhs=ones_col[0:rk, 0:1],
                        start=(ci == 0), stop=(ci == len(acols) - 1))
                r_sb = spool.tile([1, 1], f32, name=f"r_{side}_{b}", tag="r", bufs=2)
                nc.vector.reciprocal(r_sb[0:1, 0:1], s_ps[0:1, 0:1])
                att_sb = opool.tile([1, H], f32, name=f"attsb_{side}_{b}",
                                    tag="att")
                for oc in range(2):
                    att_ps = papool.tile([1, OC], f32,
                                         name=f"att_{side}_{b}_{oc}", tag="attps")
                    for ci, (acol, rk) in enumerate(acols):
                        nc.tensor.matmul(
                            att_ps[0:1, :],
                            lhsT=acol[0:rk, 0:1],
                            rhs=xn_ts[ci][0:rk, oc * OC:(oc + 1) * OC],
                            start=(ci == 0), stop=(ci == len(acols) - 1))
                    nc.scalar.activation(att_sb[0:1, oc * OC:(oc + 1) * OC],
                                         att_ps[0:1, :],
                                         Act.Copy, scale=r_sb[0:1, 0:1])
                nc.sync.dma_start(out=out_rows[b, sd:sd + 1, :],
                                  in_=att_sb[0:1, :])

            pending = None
            for b in range(BLOC):
                for side in ("img", "dns"):
                    state = emit_proj(b, side)
                    if pending is not None:
                        emit_reduce(pending)
                    pending = state
            emit_reduce(pending)
    nc.compile()
    return nc


def _get_nc():
    if "nc" not in _CACHE:
        _CACHE["nc"] = build_nc()
    return _CACHE["nc"]


def _split_e4m3(a):
    hi = a.astype(_E4M3)
    lo = (a - hi.astype(np.float32)).astype(_E4M3)
    return hi, lo


def make_in_maps(inputs):
    dns = np.ascontiguousarray(np.asarray(inputs["dns_feature"], dtype=np.float32))
    img = np.ascontiguousarray(np.asarray(inputs["img_features"], dtype=np.float32))
    W_i1 = np.asarray(inputs["W_i1"], dtype=np.float32)
    W_d2 = np.asarray(inputs["W_d2"], dtype=np.float32)
    wB = np.asarray(inputs["w_att1"], dtype=np.float32)[H:]
    wD = np.asarray(inputs["w_att2"], dtype=np.float32)[H:]

    def pack_w(W):
        hi, lo = _split_e4m3(np.ascontiguousarray(W.T) * WSCALE)
        w = np.stack([hi, lo], axis=1)              # [H, 2, H]
        return np.ascontiguousarray(
            w.reshape(HC, 128, 2, H))
    wt_i1 = pack_w(W_i1)
    wt_d2 = pack_w(W_d2)
    wrow_b = np.ascontiguousarray(np.broadcast_to(wB, (128, H)))
    wrow_d = np.ascontiguousarray(np.broadcast_to(wD, (128, H)))

    xt_dns_f = np.ascontiguousarray(
        dns.transpose(0, 2, 1).reshape(B, HC, 128, S))
    xt_img_f = np.ascontiguousarray(
        img.transpose(0, 2, 1).reshape(B, HC, 128, R))
    xt_dns = xt_dns_f.astype(_E4M3)
    xt_img = xt_img_f.astype(_E4M3)
    n_xlo = {"D": 0, "H": 2, "B": 4}[SCHEME]
    if n_xlo:
        xl_dns = (xt_dns_f - xt_dns.astype(np.float32)).astype(_E4M3)
        xl_img = (xt_img_f - xt_img.astype(np.float32)).astype(_E4M3)
    xn_dns = dns.astype(_BF16)
    xn_img = np.zeros((B, 256, H), dtype=_BF16)
    xn_img[:, :R, :] = img.astype(_BF16)

    in_maps = []
    for k in range(NCORES):
        sl = slice(k * BLOC, (k + 1) * BLOC)
        m = {
            "xt_dns": np.ascontiguousarray(xt_dns[sl]),
            "xt_img": np.ascontiguousarray(xt_img[sl]),
            "xn_dns": np.ascontiguousarray(xn_dns[sl]),
            "xn_img": np.ascontiguousarray(xn_img[sl]),
            "wt_i1": wt_i1,
            "wt_d2": wt_d2,
            "wrow_b": wrow_b,
            "wrow_d": wrow_d,
        }
        if n_xlo:
            m["xl_dns"] = np.ascontiguousarray(xl_dns[sl])
            m["xl_img"] = np.ascontiguousarray(xl_img[sl])
        in_maps.append(m)
    return in_maps


def kernel(**inputs):
    from concourse.bass_utils import run_bass_kernel_spmd

    nc = _get_nc()
    in_maps = make_in_maps(inputs)
    res = run_bass_kernel_spmd(nc, in_maps, list(range(NCORES))).results
    rows = np.concatenate([res[k]["out_rows"] for k in range(NCORES)], axis=0)
    att_img = np.ascontiguousarray(
        np.broadcast_to(rows[:, 0][:, None, :], (B, S, H)))
    att_dns = np.ascontiguousarray(
        np.broadcast_to(rows[:, 1][:, None, :], (B, S, H)))
    return att_dns, att_img


# revision 4
# speedup vs baseline: 1.1292x; 1.0062x over previous
"""CoAttention ImageDNS kernel for Trainium2 (8 NeuronCores, Bass/Tile).

Math: the reference computes two additive-attention blocks. In both, the
softmax'd score is  score[b, q, k] = f(q-side)[b, q] + g(k-side)[b, k] + c,
and softmax over k is invariant to the q-dependent (and constant) terms, so
the attention weights are independent of the query index:

  visual_att[b, s, :]  = softmax_r( wB . tanh(W_i1 @ img[b, r]) )
  textual_att[b, i, :] = softmax_j( wD . tanh(W_d2 @ dns[b, j]) )

Hence both outputs are per-batch rank-1 broadcasts:

  att_img_features[b, s, :] = visual_att[b]  @ img[b]   (same for all s)
  att_dns_features[b, i, :] = textual_att[b] @ dns[b]   (same for all i)

W_d1/b_d1/w_att1[:H]/b_att1/W_i2/b_i2/w_att2[:H]/b_att2 cancel entirely.

Sharding: pure data-parallel over batch, 4 batches per core, no collectives.

Perf notes vs the bf16 baseline (140.2us):
- Projections run as fp8(e4m3) DoubleRow matmuls (2x PE throughput). To keep
  accuracy, the DoubleRow pair slots carry a Kahan-style weight split:
  pair = (W_hi, x) + (W_lo, x) with W_hi = e4m3(64*W), W_lo = e4m3(64*W-W_hi),
  so the weight operand is ~bf16-exact and only x carries e4m3 rounding
  (end-to-end rel err ~1.7e-2 vs the 2e-2 gate; the 'B' scheme also splits x
  and lands at ~1.8e-3 at 1.5x the matmul cost — switch SCHEME if needed).
- The tiny score-sum and stage-2 matmuls for item k are emitted AFTER all of
  item k+1's projection matmuls, so the PE stream never waits on the
  scalar/vector tanh/score chain (PE idle gaps re-throttle the HAM clock
  gate from 2.4GHz to 1.2GHz, which is what capped the baseline).
- Only one [1, H] output row per (batch, side) leaves the device; the
  broadcast over S is done on host (kills 16MB/core of output DMA).
- Stage 2 (attention-weighted sum of rows) stays bf16: e4m3 there would put
  ~3.6% error directly on the output.
"""

import sys
import numpy as np
import ml_dtypes

_BF16 = ml_dtypes.bfloat16
_E4M3 = ml_dtypes.float8_e4m3

for _p in ("/opt/trn_rl_repo", "/root/.axon_site/_ro/trn_rl_repo"):
    if _p not in sys.path:
        sys.path.append(_p)

B, S, R, H = 32, 512, 196, 1024
NCORES = 8
BLOC = B // NCORES          # batches per core
OC = 512                    # output-chunk (one fp32 PSUM bank)
HC = H // 128               # 128-row contraction blocks
WSCALE = 64.0               # W quantization pre-scale (undone in tanh's scale)

# 'D': W split hi+lo, x plain e4m3  (fastest, rel err ~1.7e-2)
# 'H': D + x residual correction on blocks 0..3 (~1.2e-2, +25% matmul cost)
# 'B': D + x residual correction on all blocks  (~1.8e-3, +50% matmul cost)
SCHEME = "D"

_CACHE = {}


def _row_chunks(n):
    out, o = [], 0
    while o < n:
        out.append((o, min(128, n - o)))
        o += 128
    return out


def build_nc():
    from concourse import bacc, mybir
    from concourse import tile

    f32, f16, f8 = mybir.dt.float32, mybir.dt.bfloat16, mybir.dt.float8e4
    Act = mybir.ActivationFunctionType
    Alu = mybir.AluOpType
    DR = mybir.MatmulPerfMode.DoubleRow

    nc = bacc.Bacc("TRN2", target_bir_lowering=False, debug=False)

    n_xlo = {"D": 0, "H": 2, "B": 4}[SCHEME]  # x-residual block-pairs per oc

    RP = 256  # img row count padded to a partition multiple for single-DMA loads
    xt_dns = nc.dram_tensor("xt_dns", [BLOC, HC, 128, S], f8, kind="ExternalInput")
    xt_img = nc.dram_tensor("xt_img", [BLOC, HC, 128, R], f8, kind="ExternalInput")
    xn_dns = nc.dram_tensor("xn_dns", [BLOC, S, H], f16, kind="ExternalInput")
    xn_img = nc.dram_tensor("xn_img", [BLOC, RP, H], f16, kind="ExternalInput")
    wt_i1 = nc.dram_tensor("wt_i1", [HC, 128, H, 2], f8, kind="ExternalInput")
    wt_d2 = nc.dram_tensor("wt_d2", [HC, 128, H, 2], f8, kind="ExternalInput")
    wrow_b = nc.dram_tensor("wrow_b", [128, H], f32, kind="ExternalInput")
    wrow_d = nc.dram_tensor("wrow_d", [128, H], f32, kind="ExternalInput")
    if n_xlo:
        xl_dns = nc.dram_tensor("xl_dns", [BLOC, HC, 128, S], f8, kind="ExternalInput")
        xl_img = nc.dram_tensor("xl_img", [BLOC, HC, 128, R], f8, kind="ExternalInput")
    out_rows = nc.dram_tensor("out_rows", [BLOC, 2, H], f32, kind="ExternalOutput")

    with tile.TileContext(nc) as tc:
        with (
            tc.tile_pool(name="const", bufs=1) as cpool,
            tc.tile_pool(name="xts", bufs=2) as xtpool,
            tc.tile_pool(name="xns", bufs=2) as xnpool,
            tc.tile_pool(name="work", bufs=3) as wpool,
            tc.tile_pool(name="small", bufs=2) as spool,
            tc.tile_pool(name="outs", bufs=2) as opool,
            tc.tile_pool(name="pp", bufs=2, space="PSUM") as ppool,
            tc.tile_pool(name="pa", bufs=2, space="PSUM") as papool,
            tc.tile_pool(name="ps", bufs=2, space="PSUM") as pstat,
        ):
            wt_sb, wrow_sb = {}, {}

            def get_wrow(nm):
                if nm not in wrow_sb:
                    dram = {"b": wrow_b, "d": wrow_d}[nm]
                    w = cpool.tile([128, H], f32, name=f"wrow_{nm}_sb")
                    nc.sync.dma_start(out=w[:, :], in_=dram[:, :])
                    wrow_sb[nm] = w
                return wrow_sb[nm]

            ones_col = cpool.tile([128, 1], f16, name="ones_col")
            nc.vector.memset(ones_col[:, :], 1.0)

            # state of a finished projection phase, consumed one item later
            def emit_proj(b, side):
                n_rows = R if side == "img" else S
                xt_d = xt_img if side == "img" else xt_dns
                xn_d = xn_img if side == "img" else xn_dns
                wt_name = "i1" if side == "img" else "d2"
                load_wt = wt_name not in wt_sb
                if load_wt:
                    w = cpool.tile([128, HC * 2 * H], f8, name=f"wt_{wt_name}_sb")
                    wt_sb[wt_name] = w
                wt = wt_sb[wt_name]
                # [128, hc, var, o] view of the weight tile
                wv = wt.rearrange("p (hc o v) -> p hc v o", hc=HC, v=2)
                rcs = _row_chunks(n_rows)

                # -- loads: on a weight's first use, interleave per-hc wt/xt
                # chunks so the first projection group's deps land first --
                xt_t = xtpool.tile([128, HC * n_rows], f8,
                                   name=f"xt_{side}_{b}", tag=f"xt_{side}")
                wt_dram = {"i1": wt_i1, "d2": wt_d2}[wt_name]
                if load_wt:
                    for hc in range(HC):
                        nc.sync.dma_start(
                            out=wv[:, hc].rearrange("p v o -> p o v"),
                            in_=wt_dram[hc])
                        nc.sync.dma_start(
                            out=xt_t[:, hc * n_rows:(hc + 1) * n_rows],
                            in_=xt_d[b, hc])
                else:
                    nc.sync.dma_start(
                        out=xt_t.rearrange("p (hc m) -> p hc m", hc=HC),
                        in_=xt_d[b].rearrange("hc p m -> p hc m"))
                xv = xt_t.rearrange("p (hc m) -> p hc m", hc=HC)
                if n_xlo:
                    xl_d = xl_img if side == "img" else xl_dns
                    xl_t = xtpool.tile([128, HC * n_rows], f8,
                                       name=f"xl_{side}_{b}", tag=f"xl_{side}")
                    nc.sync.dma_start(
                        out=xl_t.rearrange("p (hc m) -> p hc m", hc=HC),
                        in_=xl_d[b].rearrange("hc p m -> p hc m"))
                    xlv = xl_t.rearrange("p (hc m) -> p hc m", hc=HC)

                # -- projection (fp8 DoubleRow), tanh, weighted o-reduce, exp --
                acols = []
                xn_ts = []
                wr = None
                for ci, (r0, rk) in enumerate(rcs):
                    ps = ppool.tile([128, H], f32, name=f"proj_{side}_{ci}_{b}",
                                    tag="pp")
                    for hc in range(HC):
                        lhs = xv[:, hc:hc + 1, r0:r0 + rk].to_broadcast(
                            (128, 2, rk))
                        for oc in range(2):
                            nc.tensor.matmul(
                                ps[0:rk, oc * OC:(oc + 1) * OC],
                                lhsT=lhs,
                                rhs=wv[:, hc, :, oc * OC:(oc + 1) * OC],
                                start=(hc == 0), stop=(hc == HC - 1 and not n_xlo),
                                perf_mode=DR)
                    for c in range(n_xlo):
                        lhs = xlv[:, 2 * c:2 * c + 2, r0:r0 + rk]
                        for oc in range(2):
                            nc.tensor.matmul(
                                ps[0:rk, oc * OC:(oc + 1) * OC],
                                lhsT=lhs,
                                rhs=wv[:, 2 * c:2 * c + 2, 0, oc * OC:(oc + 1) * OC],
                                start=False, stop=(c == n_xlo - 1),
                                perf_mode=DR)
                    if ci == 0:
                        nrc = len(rcs)
                        xn_t = xnpool.tile([128, nrc * H], f16,
                                           name=f"xn_{side}_{b}", tag=f"xn_{side}")
                        nc.sync.dma_start(
                            out=xn_t.rearrange("p (rc n) -> p rc n", rc=nrc),
                            in_=xn_d[b, 0:nrc * 128, :]
                            .rearrange("(rc p) n -> p rc n", p=128))
                        xn_ts = [xn_t[:, cj * H:(cj + 1) * H] for cj in range(nrc)]
                        wr = get_wrow("b" if side == "img" else "d")
                    th = wpool.tile([128, H], f32, name=f"th_{side}_{ci}_{b}", tag="th")
                    nc.scalar.activation(th[0:rk, :], ps[0:rk, :], Act.Tanh,
                                         scale=1.0 / WSCALE)
                    scr = wpool.tile([128, H], f32, name=f"scr_{side}_{ci}_{b}",
                                     tag="scr", bufs=2)
                    tcol = spool.tile([128, 1], f32, name=f"tc_{side}_{ci}_{b}",
                                      tag="tcol", bufs=3)
                    nc.vector.scalar_tensor_tensor(
                        out=scr[0:rk, :], in0=th[0:rk, :], scalar=1.0,
                        in1=wr[0:rk, :], op0=Alu.mult, op1=Alu.mult,
                        accum_out=tcol[0:rk, :])
                    acol = spool.tile([128, 1], f16, name=f"a_{side}_{ci}_{b}",
                                      tag=f"acol_{side}_{ci}", bufs=2)
                    nc.scalar.activation(acol[0:rk, :], tcol[0:rk, :], Act.Exp)
                    acols.append((acol, rk))
                return (b, side, acols, xn_ts)

            # sum-of-exps, reciprocal, stage-2 weighted sum, output row DMA
            def emit_reduce(state):
                b, side, acols, xn_ts = state
                sd = 0 if side == "img" else 1
                s_ps = pstat.tile([1, 1], f32, name=f"s_{side}_{b}", tag="stat")
                for ci, (acol, rk) in enumerate(acols):
                    nc.tensor.matmul(
                        s_ps[0:1, 0:1], lhsT=acol[0:rk, 0:1],
                        rhs=ones_col[0:rk, 0:1],
                        start=(ci == 0), stop=(ci == len(acols) - 1))
                r_sb = spool.tile([1, 1], f32, name=f"r_{side}_{b}", tag="r", bufs=2)
                nc.vector.reciprocal(r_sb[0:1, 0:1], s_ps[0:1, 0:1])
                att_sb = opool.tile([1, H], f32, name=f"attsb_{side}_{b}",
                                    tag="att")
                for oc in range(2):
                    att_ps = papool.tile([1, OC], f32,
                                         name=f"att_{side}_{b}_{oc}", tag="attps")
                    for ci, (acol, rk) in enumerate(acols):
                        nc.tensor.matmul(
                            att_ps[0:1, :],
                            lhsT=acol[0:rk, 0:1],
                            rhs=xn_ts[ci][0:rk, oc * OC:(oc + 1) * OC],
                            start=(ci == 0), stop=(ci == len(acols) - 1))
                    nc.scalar.activation(att_sb[0:1, oc * OC:(oc + 1) * OC],
                                         att_ps[0:1, :],
                                         Act.Copy, scale=r_sb[0:1, 0:1])
                nc.sync.dma_start(out=out_rows[b, sd:sd + 1, :],
                                  in_=att_sb[0:1, :])

            pending = None
            for b in range(BLOC):
                for side in ("img", "dns"):
                    state = emit_proj(b, side)
                    if pending is not None:
                        emit_reduce(pending)
                    pending = state
            emit_reduce(pending)
    nc.compile()
    return nc


def _get_nc():
    if "nc" not in _CACHE:
        _CACHE["nc"] = build_nc()
    return _CACHE["nc"]


def _split_e4m3(a):
    hi = a.astype(_E4M3)
    lo = (a - hi.astype(np.float32)).astype(_E4M3)
    return hi, lo


def make_in_maps(inputs):
    dns = np.ascontiguousarray(np.asarray(inputs["dns_feature"], dtype=np.float32))
    img = np.ascontiguousarray(np.asarray(inputs["img_features"], dtype=np.float32))
    W_i1 = np.asarray(inputs["W_i1"], dtype=np.float32)
    W_d2 = np.asarray(inputs["W_d2"], dtype=np.float32)
    wB = np.asarray(inputs["w_att1"], dtype=np.float32)[H:]
    wD = np.asarray(inputs["w_att2"], dtype=np.float32)[H:]

    def pack_w(W):
        hi, lo = _split_e4m3(np.ascontiguousarray(W.T) * WSCALE)
        w = np.stack([hi, lo], axis=-1)             # [H, H, 2] (v innermost)
        return np.ascontiguousarray(
            w.reshape(HC, 128, H, 2))
    wt_i1 = pack_w(W_i1)
    wt_d2 = pack_w(W_d2)
    wrow_b = np.ascontiguousarray(np.broadcast_to(wB, (128, H)))
    wrow_d = np.ascontiguousarray(np.broadcast_to(wD, (128, H)))

    xt_dns_f = np.ascontiguousarray(
        dns.transpose(0, 2, 1).reshape(B, HC, 128, S))
    xt_img_f = np.ascontiguousarray(
        img.transpose(0, 2, 1).reshape(B, HC, 128, R))
    xt_dns = xt_dns_f.astype(_E4M3)
    xt_img = xt_img_f.astype(_E4M3)
    n_xlo = {"D": 0, "H": 2, "B": 4}[SCHEME]
    if n_xlo:
        xl_dns = (xt_dns_f - xt_dns.astype(np.float32)).astype(_E4M3)
        xl_img = (xt_img_f - xt_img.astype(np.float32)).astype(_E4M3)
    xn_dns = dns.astype(_BF16)
    xn_img = np.zeros((B, 256, H), dtype=_BF16)
    xn_img[:, :R, :] = img.astype(_BF16)

    in_maps = []
    for k in range(NCORES):
        sl = slice(k * BLOC, (k + 1) * BLOC)
        m = {
            "xt_dns": np.ascontiguousarray(xt_dns[sl]),
            "xt_img": np.ascontiguousarray(xt_img[sl]),
            "xn_dns": np.ascontiguousarray(xn_dns[sl]),
            "xn_img": np.ascontiguousarray(xn_img[sl]),
            "wt_i1": wt_i1,
            "wt_d2": wt_d2,
            "wrow_b": wrow_b,
            "wrow_d": wrow_d,
        }
        if n_xlo:
            m["xl_dns"] = np.ascontiguousarray(xl_dns[sl])
            m["xl_img"] = np.ascontiguousarray(xl_img[sl])
        in_maps.append(m)
    return in_maps


def kernel(**inputs):
    from concourse.bass_utils import run_bass_kernel_spmd

    nc = _get_nc()
    in_maps = make_in_maps(inputs)
    res = run_bass_kernel_spmd(nc, in_maps, list(range(NCORES))).results
    rows = np.concatenate([res[k]["out_rows"] for k in range(NCORES)], axis=0)
    att_img = np.ascontiguousarray(
        np.broadcast_to(rows[:, 0][:, None, :], (B, S, H)))
    att_dns = np.ascontiguousarray(
        np.broadcast_to(rows[:, 1][:, None, :], (B, S, H)))
    return att_dns, att_img


# revision 6
# speedup vs baseline: 1.2159x; 1.0768x over previous
"""CoAttention ImageDNS kernel for Trainium2 (8 NeuronCores, Bass/Tile).

Math: the reference computes two additive-attention blocks. In both, the
softmax'd score is  score[b, q, k] = f(q-side)[b, q] + g(k-side)[b, k] + c,
and softmax over k is invariant to the q-dependent (and constant) terms, so
the attention weights are independent of the query index:

  visual_att[b, s, :]  = softmax_r( wB . tanh(W_i1 @ img[b, r]) )
  textual_att[b, i, :] = softmax_j( wD . tanh(W_d2 @ dns[b, j]) )

Hence both outputs are per-batch rank-1 broadcasts:

  att_img_features[b, s, :] = visual_att[b]  @ img[b]   (same for all s)
  att_dns_features[b, i, :] = textual_att[b] @ dns[b]   (same for all i)

W_d1/b_d1/w_att1[:H]/b_att1/W_i2/b_i2/w_att2[:H]/b_att2 cancel entirely.

Sharding: pure data-parallel over batch, 4 batches per core, no collectives.

Perf notes vs the bf16 baseline (140.2us):
- Projection h-blocks 0..3 run as fp8(e4m3) DoubleRow matmuls with a REAL
  256-deep contraction per matmul (2 h-blocks per pair-column), which the PE
  streams at the same column rate as a 128-deep bf16 matmul -> 2x throughput
  on that half. Blocks 4..7 stay bf16. Net projection cost 0.75x, end-to-end
  rel err ~1.77e-2 vs the 2e-2 gate (fp8 on ALL blocks would be 2.6e-2; W is
  pre-scaled by 64 so its entries clear e4m3's subnormal floor).
- The tiny score-sum and stage-2 matmuls for item k are emitted AFTER all of
  item k+1's projection matmuls, so the PE stream never waits on the
  scalar/vector tanh/score chain (PE idle gaps re-throttle the HAM clock
  gate from 2.4GHz to 1.2GHz, which is what capped the baseline).
- Inputs stream on three DMA queues (weights on GpSimd's, x^T tiles on
  Sync's, stage-2 activations on Activation's) instead of one.
- Only one [1, H] output row per (batch, side) leaves the device; the
  broadcast over S is done on host (kills 16MB/core of output DMA).
- Stage 2 (attention-weighted sum of rows) stays bf16: e4m3 there would put
  ~3.6% error directly on the output.
"""

import sys
import numpy as np
import ml_dtypes

_BF16 = ml_dtypes.bfloat16
_E4M3 = ml_dtypes.float8_e4m3

for _p in ("/opt/trn_rl_repo", "/root/.axon_site/_ro/trn_rl_repo"):
    if _p not in sys.path:
        sys.path.append(_p)

B, S, R, H = 32, 512, 196, 1024
NCORES = 8
BLOC = B // NCORES          # batches per core
OC = 512                    # output-chunk (one fp32 PSUM bank)
NB8 = 4                     # h-blocks 0..3 in e4m3 (2 DoubleRow matmuls)
NBB = 4                     # h-blocks 4..7 in bf16
WSCALE = 64.0               # W pre-scale so e4m3 entries are normal numbers

_CACHE = {}


def _row_chunks(n):
    out, o = [], 0
    while o < n:
        out.append((o, min(128, n - o)))
        o += 128
    return out


def build_nc():
    from concourse import bacc, mybir
    from concourse import tile

    f32, f16, f8 = mybir.dt.float32, mybir.dt.bfloat16, mybir.dt.float8e4
    Act = mybir.ActivationFunctionType
    Alu = mybir.AluOpType
    DR = mybir.MatmulPerfMode.DoubleRow

    nc = bacc.Bacc("TRN2", target_bir_lowering=False, debug=False)

    RP = 256  # img row count padded to a partition multiple for single-DMA loads
    R8 = 208  # img rows padded so the DoubleRow pair-dim step is 16B-aligned
    x8_dns = nc.dram_tensor("x8_dns", [BLOC, NB8, 128, S], f8, kind="ExternalInput")
    x8_img = nc.dram_tensor("x8_img", [BLOC, NB8, 128, R8], f8, kind="ExternalInput")
    xb_dns = nc.dram_tensor("xb_dns", [BLOC, NBB, 128, S], f16, kind="ExternalInput")
    xb_img = nc.dram_tensor("xb_img", [BLOC, NBB, 128, R], f16, kind="ExternalInput")
    xn_dns = nc.dram_tensor("xn_dns", [BLOC, S, H], f16, kind="ExternalInput")
    xn_img = nc.dram_tensor("xn_img", [BLOC, RP, H], f16, kind="ExternalInput")
    w8_i1 = nc.dram_tensor("w8_i1", [NB8, 128, H], f8, kind="ExternalInput")
    wb_i1 = nc.dram_tensor("wb_i1", [NBB, 128, H], f16, kind="ExternalInput")
    w8_d2 = nc.dram_tensor("w8_d2", [NB8, 128, H], f8, kind="ExternalInput")
    wb_d2 = nc.dram_tensor("wb_d2", [NBB, 128, H], f16, kind="ExternalInput")
    wrow_b = nc.dram_tensor("wrow_b", [128, H], f32, kind="ExternalInput")
    wrow_d = nc.dram_tensor("wrow_d", [128, H], f32, kind="ExternalInput")
    out_rows = nc.dram_tensor("out_rows", [BLOC, 2, H], f32, kind="ExternalOutput")

    with tile.TileContext(nc) as tc:
        with (
            tc.tile_pool(name="const", bufs=1) as cpool,
            tc.tile_pool(name="xts", bufs=2) as xtpool,
            tc.tile_pool(name="xns", bufs=2) as xnpool,
            tc.tile_pool(name="work", bufs=3) as wpool,
            tc.tile_pool(name="small", bufs=2) as spool,
            tc.tile_pool(name="outs", bufs=2) as opool,
            tc.tile_pool(name="pp", bufs=2, space="PSUM") as ppool,
            tc.tile_pool(name="pa", bufs=2, space="PSUM") as papool,
            tc.tile_pool(name="ps", bufs=2, space="PSUM") as pstat,
        ):
            wt_sb, wrow_sb = {}, {}

            def get_wrow(nm):
                if nm not in wrow_sb:
                    dram = {"b": wrow_b, "d": wrow_d}[nm]
                    w = cpool.tile([128, H], f32, name=f"wrow_{nm}_sb")
                    nc.gpsimd.dma_start(out=w[:, :], in_=dram[:, :])
                    wrow_sb[nm] = w
                return wrow_sb[nm]

            ones_col = cpool.tile([128, 1], f16, name="ones_col")
            nc.vector.memset(ones_col[:, :], 1.0)

            def emit_proj(b, side):
                n_rows = R if side == "img" else S
                n8 = R8 if side == "img" else S
                x8_d = x8_img if side == "img" else x8_dns
                xb_d = xb_img if side == "img" else xb_dns
                xn_d = xn_img if side == "img" else xn_dns
                wt_name = "i1" if side == "img" else "d2"
                if wt_name not in wt_sb:
                    w8_d, wb_d = (w8_i1, wb_i1) if side == "img" else (w8_d2, wb_d2)
                    w8 = cpool.tile([128, NB8 * H], f8, name=f"w8_{wt_name}_sb")
                    wb = cpool.tile([128, NBB * H], f16, name=f"wb_{wt_name}_sb")
                    # weights stream on the (otherwise idle) GpSimd DMA queue,
                    # per-block in consumption order
                    for j in range(NB8):
                        nc.gpsimd.dma_start(out=w8[:, j * H:(j + 1) * H],
                                            in_=w8_d[j])
                    for j in range(NBB):
                        nc.gpsimd.dma_start(out=wb[:, j * H:(j + 1) * H],
                                            in_=wb_d[j])
                    wt_sb[wt_name] = (w8, wb)
                w8, wb = wt_sb[wt_name]
                w8v = w8.rearrange("p (j o) -> p j o", j=NB8)
                rcs = _row_chunks(n_rows)

                x8_t = xtpool.tile([128, NB8 * n8], f8,
                                   name=f"x8_{side}_{b}", tag=f"x8_{side}")
                nc.sync.dma_start(
                    out=x8_t.rearrange("p (j m) -> p j m", j=NB8),
                    in_=x8_d[b].rearrange("j p m -> p j m"))
                xb_t = xtpool.tile([128, NBB * n_rows], f16,
                                   name=f"xb_{side}_{b}", tag=f"xb_{side}")
                nc.sync.dma_start(
                    out=xb_t.rearrange("p (j m) -> p j m", j=NBB),
                    in_=xb_d[b].rearrange("j p m -> p j m"))
                x8v = x8_t.rearrange("p (j m) -> p j m", j=NB8)  # j-stride n8

                acols = []
                xn_ts = []
                wr = None
                for ci, (r0, rk) in enumerate(rcs):
                    ps = ppool.tile([128, H], f32, name=f"proj_{side}_{ci}_{b}",
                                    tag="pp")
                    for u in range(NB8 // 2):
                        lhs = x8v[:, 2 * u:2 * u + 2, r0:r0 + rk]
                        for oc in range(2):
                            nc.tensor.matmul(
                                ps[0:rk, oc * OC:(oc + 1) * OC],
                                lhsT=lhs,
                                rhs=w8v[:, 2 * u:2 * u + 2, oc * OC:(oc + 1) * OC],
                                start=(u == 0), stop=False,
                                perf_mode=DR)
                    for j in range(NBB):
                        lhs = xb_t[:, j * n_rows + r0: j * n_rows + r0 + rk]
                        for oc in range(2):
                            nc.tensor.matmul(
                                ps[0:rk, oc * OC:(oc + 1) * OC],
                                lhsT=lhs,
                                rhs=wb[:, j * H + oc * OC: j * H + (oc + 1) * OC],
                                start=False, stop=(j == NBB - 1))
                    if ci == 0:
                        nrc = len(rcs)
                        xn_t = xnpool.tile([128, nrc * H], f16,
                                           name=f"xn_{side}_{b}", tag=f"xn_{side}")
                        # stage-2 activations stream on the Activation queue;
                        # they are consumed one pipeline item later
                        nc.scalar.dma_start(
                            out=xn_t.rearrange("p (rc n) -> p rc n", rc=nrc),
                            in_=xn_d[b, 0:nrc * 128, :]
                            .rearrange("(rc p) n -> p rc n", p=128))
                        xn_ts = [xn_t[:, cj * H:(cj + 1) * H] for cj in range(nrc)]
                        wr = get_wrow("b" if side == "img" else "d")
                    th = wpool.tile([128, H], f32, name=f"th_{side}_{ci}_{b}", tag="th")
                    nc.scalar.activation(th[0:rk, :], ps[0:rk, :], Act.Tanh,
                                         scale=1.0 / WSCALE)
                    scr = wpool.tile([128, H], f32, name=f"scr_{side}_{ci}_{b}",
                                     tag="scr", bufs=2)
                    tcol = spool.tile([128, 1], f32, name=f"tc_{side}_{ci}_{b}",
                                      tag="tcol", bufs=3)
                    nc.vector.scalar_tensor_tensor(
                        out=scr[0:rk, :], in0=th[0:rk, :], scalar=1.0,
                        in1=wr[0:rk, :], op0=Alu.mult, op1=Alu.mult,
                        accum_out=tcol[0:rk, :])
                    acol = spool.tile([128, 1], f16, name=f"a_{side}_{ci}_{b}",
                                      tag=f"acol_{side}_{ci}", bufs=2)
                    nc.scalar.activation(acol[0:rk, :], tcol[0:rk, :], Act.Exp)
                    acols.append((acol, rk))
                return (b, side, acols, xn_ts)

            def emit_reduce(state):
                b, side, acols, xn_ts = state
                sd = 0 if side == "img" else 1
                s_ps = pstat.tile([1, 1], f32, name=f"s_{side}_{b}", tag="stat")
                for ci, (acol, rk) in enumerate(acols):
                    nc.tensor.matmul(
                        s_ps[0:1, 0:1], lhsT=acol[0:rk, 0:1],
                        rhs=ones_col[0:rk, 0:1],
                        start=(ci == 0), stop=(ci == len(acols) - 1))
                r_sb = spool.tile([1, 1], f32, name=f"r_{side}_{b}", tag="r", bufs=2)
                nc.vector.reciprocal(r_sb[0:1, 0:1], s_ps[0:1, 0:1])
                att_sb = opool.tile([1, H], f32, name=f"attsb_{side}_{b}",
                                    tag="att")
                for oc in range(2):
                    att_ps = papool.tile([1, OC], f32,
                                         name=f"att_{side}_{b}_{oc}", tag="attps")
                    for ci, (acol, rk) in enumerate(acols):
                        nc.tensor.matmul(
                            att_ps[0:1, :],
                            lhsT=acol[0:rk, 0:1],
                            rhs=xn_ts[ci][0:rk, oc * OC:(oc + 1) * OC],
                            start=(ci == 0), stop=(ci == len(acols) - 1))
                    nc.scalar.activation(att_sb[0:1, oc * OC:(oc + 1) * OC],
                                         att_ps[0:1, :],
                                         Act.Copy, scale=r_sb[0:1, 0:1])
                nc.sync.dma_start(out=out_rows[b, sd:sd + 1, :],
                                  in_=att_sb[0:1, :])

            pending = None
            for b in range(BLOC):
                for side in ("img", "dns"):
                    state = emit_proj(b, side)
                    if pending is not None:
                        emit_reduce(pending)
                    pending = state
            emit_reduce(pending)
    nc.compile()
    return nc


def _get_nc():
    if "nc" not in _CACHE:
        _CACHE["nc"] = build_nc()
    return _CACHE["nc"]


def make_in_maps(inputs):
    dns = np.ascontiguousarray(np.asarray(inputs["dns_feature"], dtype=np.float32))
    img = np.ascontiguousarray(np.asarray(inputs["img_features"], dtype=np.float32))
    W_i1 = np.asarray(inputs["W_i1"], dtype=np.float32)
    W_d2 = np.asarray(inputs["W_d2"], dtype=np.float32)
    wB = np.asarray(inputs["w_att1"], dtype=np.float32)[H:]
    wD = np.asarray(inputs["w_att2"], dtype=np.float32)[H:]

    def pack_w(W):
        wt = np.ascontiguousarray(W.T) * WSCALE         # [h_in, o]
        w8 = np.ascontiguousarray(wt[:NB8 * 128].reshape(NB8, 128, H)).astype(_E4M3)
        wb = np.ascontiguousarray(wt[NB8 * 128:].reshape(NBB, 128, H)).astype(_BF16)
        return w8, wb
    w8_i1, wb_i1 = pack_w(W_i1)
    w8_d2, wb_d2 = pack_w(W_d2)
    wrow_b = np.ascontiguousarray(np.broadcast_to(wB, (128, H)))
    wrow_d = np.ascontiguousarray(np.broadcast_to(wD, (128, H)))

    def pack_x(x, n, n8):
        xt = np.ascontiguousarray(x.transpose(0, 2, 1).reshape(B, 8, 128, n))
        x8 = np.zeros((B, NB8, 128, n8), dtype=_E4M3)
        x8[:, :, :, :n] = xt[:, :NB8].astype(_E4M3)
        xb = np.ascontiguousarray(xt[:, NB8:]).astype(_BF16)
        return x8, xb
    x8_dns, xb_dns = pack_x(dns, S, S)
    x8_img, xb_img = pack_x(img, R, 208)
    xn_dns = dns.astype(_BF16)
    xn_img = np.zeros((B, 256, H), dtype=_BF16)
    xn_img[:, :R, :] = img.astype(_BF16)

    in_maps = []
    for k in range(NCORES):
        sl = slice(k * BLOC, (k + 1) * BLOC)
        in_maps.append({
            "x8_dns": np.ascontiguousarray(x8_dns[sl]),
            "x8_img": np.ascontiguousarray(x8_img[sl]),
            "xb_dns": np.ascontiguousarray(xb_dns[sl]),
            "xb_img": np.ascontiguousarray(xb_img[sl]),
            "xn_dns": np.ascontiguousarray(xn_dns[sl]),
            "xn_img": np.ascontiguousarray(xn_img[sl]),
            "w8_i1": w8_i1, "wb_i1": wb_i1,
            "w8_d2": w8_d2, "wb_d2": wb_d2,
            "wrow_b": wrow_b, "wrow_d": wrow_d,
        })
    return in_maps


def kernel(**inputs):
    from concourse.bass_utils import run_bass_kernel_spmd

    nc = _get_nc()
    in_maps = make_in_maps(inputs)
    res = run_bass_kernel_spmd(nc, in_maps, list(range(NCORES))).results
    rows = np.concatenate([res[k]["out_rows"] for k in range(NCORES)], axis=0)
    att_img = np.ascontiguousarray(
        np.broadcast_to(rows[:, 0][:, None, :], (B, S, H)))
    att_dns = np.ascontiguousarray(
        np.broadcast_to(rows[:, 1][:, None, :], (B, S, H)))
    return att_dns, att_img


# revision 7
# speedup vs baseline: 1.2467x; 1.0253x over previous
"""CoAttention ImageDNS kernel for Trainium2 (8 NeuronCores, Bass/Tile).

Math: the reference computes two additive-attention blocks. In both, the
softmax'd score is  score[b, q, k] = f(q-side)[b, q] + g(k-side)[b, k] + c,
and softmax over k is invariant to the q-dependent (and constant) terms, so
the attention weights are independent of the query index:

  visual_att[b, s, :]  = softmax_r( wB . tanh(W_i1 @ img[b, r]) )
  textual_att[b, i, :] = softmax_j( wD . tanh(W_d2 @ dns[b, j]) )

Hence both outputs are per-batch rank-1 broadcasts:

  att_img_features[b, s, :] = visual_att[b]  @ img[b]   (same for all s)
  att_dns_features[b, i, :] = textual_att[b] @ dns[b]   (same for all i)

W_d1/b_d1/w_att1[:H]/b_att1/W_i2/b_i2/w_att2[:H]/b_att2 cancel entirely.

Sharding: pure data-parallel over batch, 4 batches per core, no collectives.

Perf notes vs the bf16 baseline (140.2us):
- Projection h-blocks 0..3 run as fp8(e4m3) DoubleRow matmuls with a REAL
  256-deep contraction per matmul (2 h-blocks per pair-column), which the PE
  streams at the same column rate as a 128-deep bf16 matmul -> 2x throughput
  on that half. Blocks 4..7 stay bf16. Net projection cost 0.75x, end-to-end
  rel err ~1.77e-2 vs the 2e-2 gate (fp8 on ALL blocks would be 2.6e-2; W is
  pre-scaled by 64 so its entries clear e4m3's subnormal floor).
- The tiny score-sum and stage-2 matmuls for item k are emitted AFTER all of
  item k+1's projection matmuls, so the PE stream never waits on the
  scalar/vector tanh/score chain (PE idle gaps re-throttle the HAM clock
  gate from 2.4GHz to 1.2GHz, which is what capped the baseline).
- Inputs stream on three DMA queues (weights on GpSimd's, x^T tiles on
  Sync's, stage-2 activations on Activation's) instead of one.
- Only one [1, H] output row per (batch, side) leaves the device; the
  broadcast over S is done on host (kills 16MB/core of output DMA).
- Stage 2 (attention-weighted sum of rows) stays bf16: e4m3 there would put
  ~3.6% error directly on the output.
"""

import sys
import numpy as np
import ml_dtypes

_BF16 = ml_dtypes.bfloat16
_E4M3 = ml_dtypes.float8_e4m3

for _p in ("/opt/trn_rl_repo", "/root/.axon_site/_ro/trn_rl_repo"):
    if _p not in sys.path:
        sys.path.append(_p)

B, S, R, H = 32, 512, 196, 1024
NCORES = 8
BLOC = B // NCORES          # batches per core
OC = 512                    # output-chunk (one fp32 PSUM bank)
NB8 = 4                     # h-blocks 0..3 in e4m3 (2 DoubleRow matmuls)
NBB = 4                     # h-blocks 4..7 in bf16
WSCALE = 64.0               # W pre-scale so e4m3 entries are normal numbers

_CACHE = {}


def _row_chunks(n):
    out, o = [], 0
    while o < n:
        out.append((o, min(128, n - o)))
        o += 128
    return out


def build_nc():
    from concourse import bacc, mybir
    from concourse import tile

    f32, f16, f8 = mybir.dt.float32, mybir.dt.bfloat16, mybir.dt.float8e4
    Act = mybir.ActivationFunctionType
    Alu = mybir.AluOpType
    DR = mybir.MatmulPerfMode.DoubleRow

    nc = bacc.Bacc("TRN2", target_bir_lowering=False, debug=False)

    RP = 256  # img row count padded to a partition multiple for single-DMA loads
    R8 = 208  # img rows padded so the DoubleRow pair-dim step is 16B-aligned
    x8_dns = nc.dram_tensor("x8_dns", [BLOC, 128, NB8 * S], f8, kind="ExternalInput")
    x8_img = nc.dram_tensor("x8_img", [BLOC, 128, NB8 * R8], f8, kind="ExternalInput")
    xb_dns = nc.dram_tensor("xb_dns", [BLOC, 128, NBB * S], f16, kind="ExternalInput")
    xb_img = nc.dram_tensor("xb_img", [BLOC, 128, NBB * R], f16, kind="ExternalInput")
    xn_dns = nc.dram_tensor("xn_dns", [BLOC, S, H], f16, kind="ExternalInput")
    xn_img = nc.dram_tensor("xn_img", [BLOC, RP, H], f16, kind="ExternalInput")
    w8_i1 = nc.dram_tensor("w8_i1", [NB8, 128, H], f8, kind="ExternalInput")
    wb_i1 = nc.dram_tensor("wb_i1", [NBB, 128, H], f16, kind="ExternalInput")
    w8_d2 = nc.dram_tensor("w8_d2", [NB8, 128, H], f8, kind="ExternalInput")
    wb_d2 = nc.dram_tensor("wb_d2", [NBB, 128, H], f16, kind="ExternalInput")
    wrow_b = nc.dram_tensor("wrow_b", [128, H], f32, kind="ExternalInput")
    wrow_d = nc.dram_tensor("wrow_d", [128, H], f32, kind="ExternalInput")
    out_rows = nc.dram_tensor("out_rows", [BLOC, 2, H], f32, kind="ExternalOutput")

    with tile.TileContext(nc) as tc:
        with (
            tc.tile_pool(name="const", bufs=1) as cpool,
            tc.tile_pool(name="xts", bufs=2) as xtpool,
            tc.tile_pool(name="xns", bufs=2) as xnpool,
            tc.tile_pool(name="work", bufs=3) as wpool,
            tc.tile_pool(name="small", bufs=2) as spool,
            tc.tile_pool(name="outs", bufs=2) as opool,
            tc.tile_pool(name="pp", bufs=2, space="PSUM") as ppool,
            tc.tile_pool(name="pa", bufs=2, space="PSUM") as papool,
            tc.tile_pool(name="ps", bufs=2, space="PSUM") as pstat,
        ):
            wt_sb, wrow_sb = {}, {}

            def get_wrow(nm):
                if nm not in wrow_sb:
                    dram = {"b": wrow_b, "d": wrow_d}[nm]
                    w = cpool.tile([128, H], f32, name=f"wrow_{nm}_sb")
                    nc.gpsimd.dma_start(out=w[:, :], in_=dram[:, :])
                    wrow_sb[nm] = w
                return wrow_sb[nm]

            ones_col = cpool.tile([128, 1], f16, name="ones_col")
            nc.vector.memset(ones_col[:, :], 1.0)

            def emit_proj(b, side):
                n_rows = R if side == "img" else S
                n8 = R8 if side == "img" else S
                x8_d = x8_img if side == "img" else x8_dns
                xb_d = xb_img if side == "img" else xb_dns
                xn_d = xn_img if side == "img" else xn_dns
                wt_name = "i1" if side == "img" else "d2"
                if wt_name not in wt_sb:
                    w8_d, wb_d = (w8_i1, wb_i1) if side == "img" else (w8_d2, wb_d2)
                    w8 = cpool.tile([128, NB8 * H], f8, name=f"w8_{wt_name}_sb")
                    wb = cpool.tile([128, NBB * H], f16, name=f"wb_{wt_name}_sb")
                    # weights stream on the (otherwise idle) GpSimd DMA queue,
                    # per-block in consumption order
                    for j in range(NB8):
                        nc.gpsimd.dma_start(out=w8[:, j * H:(j + 1) * H],
                                            in_=w8_d[j])
                    for j in range(NBB):
                        nc.gpsimd.dma_start(out=wb[:, j * H:(j + 1) * H],
                                            in_=wb_d[j])
                    wt_sb[wt_name] = (w8, wb)
                w8, wb = wt_sb[wt_name]
                w8v = w8.rearrange("p (j o) -> p j o", j=NB8)
                rcs = _row_chunks(n_rows)

                x8_t = xtpool.tile([128, NB8 * n8], f8,
                                   name=f"x8_{side}_{b}", tag=f"x8_{side}")
                nc.sync.dma_start(out=x8_t[:, :], in_=x8_d[b])
                xb_t = xtpool.tile([128, NBB * n_rows], f16,
                                   name=f"xb_{side}_{b}", tag=f"xb_{side}")
                nc.sync.dma_start(out=xb_t[:, :], in_=xb_d[b])
                x8v = x8_t.rearrange("p (j m) -> p j m", j=NB8)  # j-stride n8

                acols = []
                xn_ts = []
                wr = None
                for ci, (r0, rk) in enumerate(rcs):
                    ps = ppool.tile([128, H], f32, name=f"proj_{side}_{ci}_{b}",
                                    tag="pp")
                    for u in range(NB8 // 2):
                        lhs = x8v[:, 2 * u:2 * u + 2, r0:r0 + rk]
                        for oc in range(2):
                            nc.tensor.matmul(
                                ps[0:rk, oc * OC:(oc + 1) * OC],
                                lhsT=lhs,
                                rhs=w8v[:, 2 * u:2 * u + 2, oc * OC:(oc + 1) * OC],
                                start=(u == 0), stop=False,
                                perf_mode=DR)
                    for j in range(NBB):
                        lhs = xb_t[:, j * n_rows + r0: j * n_rows + r0 + rk]
                        for oc in range(2):
                            nc.tensor.matmul(
                                ps[0:rk, oc * OC:(oc + 1) * OC],
                                lhsT=lhs,
                                rhs=wb[:, j * H + oc * OC: j * H + (oc + 1) * OC],
                                start=False, stop=(j == NBB - 1))
                    if ci == 0:
                        nrc = len(rcs)
                        xn_t = xnpool.tile([128, nrc * H], f16,
                                           name=f"xn_{side}_{b}", tag=f"xn_{side}")
                        # stage-2 activations stream on the Activation queue;
                        # they are consumed one pipeline item later
                        nc.scalar.dma_start(
                            out=xn_t.rearrange("p (rc n) -> p rc n", rc=nrc),
                            in_=xn_d[b, 0:nrc * 128, :]
                            .rearrange("(rc p) n -> p rc n", p=128))
                        xn_ts = [xn_t[:, cj * H:(cj + 1) * H] for cj in range(nrc)]
                        wr = get_wrow("b" if side == "img" else "d")
                    th = wpool.tile([128, H], f32, name=f"th_{side}_{ci}_{b}", tag="th")
                    nc.scalar.activation(th[0:rk, :], ps[0:rk, :], Act.Tanh,
                                         scale=1.0 / WSCALE)
                    scr = wpool.tile([128, H], f32, name=f"scr_{side}_{ci}_{b}",
                                     tag="scr", bufs=2)
                    tcol = spool.tile([128, 1], f32, name=f"tc_{side}_{ci}_{b}",
                                      tag="tcol", bufs=3)
                    nc.vector.scalar_tensor_tensor(
                        out=scr[0:rk, :], in0=th[0:rk, :], scalar=1.0,
                        in1=wr[0:rk, :], op0=Alu.mult, op1=Alu.mult,
                        accum_out=tcol[0:rk, :])
                    acol = spool.tile([128, 1], f16, name=f"a_{side}_{ci}_{b}",
                                      tag=f"acol_{side}_{ci}", bufs=2)
                    nc.scalar.activation(acol[0:rk, :], tcol[0:rk, :], Act.Exp)
                    acols.append((acol, rk))
                return (b, side, acols, xn_ts)

            def emit_reduce(state):
                b, side, acols, xn_ts = state
                sd = 0 if side == "img" else 1
                s_ps = pstat.tile([1, 1], f32, name=f"s_{side}_{b}", tag="stat")
                for ci, (acol, rk) in enumerate(acols):
                    nc.tensor.matmul(
                        s_ps[0:1, 0:1], lhsT=acol[0:rk, 0:1],
                        rhs=ones_col[0:rk, 0:1],
                        start=(ci == 0), stop=(ci == len(acols) - 1))
                r_sb = spool.tile([1, 1], f32, name=f"r_{side}_{b}", tag="r", bufs=2)
                nc.vector.reciprocal(r_sb[0:1, 0:1], s_ps[0:1, 0:1])
                att_sb = opool.tile([1, H], f32, name=f"attsb_{side}_{b}",
                                    tag="att")
                for oc in range(2):
                    att_ps = papool.tile([1, OC], f32,
                                         name=f"att_{side}_{b}_{oc}", tag="attps")
                    for ci, (acol, rk) in enumerate(acols):
                        nc.tensor.matmul(
                            att_ps[0:1, :],
                            lhsT=acol[0:rk, 0:1],
                            rhs=xn_ts[ci][0:rk, oc * OC:(oc + 1) * OC],
                            start=(ci == 0), stop=(ci == len(acols) - 1))
                    nc.scalar.activation(att_sb[0:1, oc * OC:(oc + 1) * OC],
                                         att_ps[0:1, :],
                                         Act.Copy, scale=r_sb[0:1, 0:1])
                nc.sync.dma_start(out=out_rows[b, sd:sd + 1, :],
                                  in_=att_sb[0:1, :])

            pending = None
            for b in range(BLOC):
                for side in ("img", "dns"):
                    state = emit_proj(b, side)
                    if pending is not None:
                        emit_reduce(pending)
                    pending = state
            emit_reduce(pending)
    nc.compile()
    return nc


def _get_nc():
    if "nc" not in _CACHE:
        _CACHE["nc"] = build_nc()
    return _CACHE["nc"]


def make_in_maps(inputs):
    dns = np.ascontiguousarray(np.asarray(inputs["dns_feature"], dtype=np.float32))
    img = np.ascontiguousarray(np.asarray(inputs["img_features"], dtype=np.float32))
    W_i1 = np.asarray(inputs["W_i1"], dtype=np.float32)
    W_d2 = np.asarray(inputs["W_d2"], dtype=np.float32)
    wB = np.asarray(inputs["w_att1"], dtype=np.float32)[H:]
    wD = np.asarray(inputs["w_att2"], dtype=np.float32)[H:]

    def pack_w(W):
        wt = np.ascontiguousarray(W.T) * WSCALE         # [h_in, o]
        w8 = np.ascontiguousarray(wt[:NB8 * 128].reshape(NB8, 128, H)).astype(_E4M3)
        wb = np.ascontiguousarray(wt[NB8 * 128:].reshape(NBB, 128, H)).astype(_BF16)
        return w8, wb
    w8_i1, wb_i1 = pack_w(W_i1)
    w8_d2, wb_d2 = pack_w(W_d2)
    wrow_b = np.ascontiguousarray(np.broadcast_to(wB, (128, H)))
    wrow_d = np.ascontiguousarray(np.broadcast_to(wD, (128, H)))

    def pack_x(x, n, n8):
        xt = x.transpose(0, 2, 1).reshape(B, 8, 128, n)
        x8 = np.zeros((B, NB8, 128, n8), dtype=_E4M3)
        x8[:, :, :, :n] = xt[:, :NB8].astype(_E4M3)
        x8 = np.ascontiguousarray(x8.transpose(0, 2, 1, 3).reshape(B, 128, NB8 * n8))
        xb = xt[:, NB8:].astype(_BF16)
        xb = np.ascontiguousarray(xb.transpose(0, 2, 1, 3).reshape(B, 128, NBB * n))
        return x8, xb
    x8_dns, xb_dns = pack_x(dns, S, S)
    x8_img, xb_img = pack_x(img, R, 208)
    xn_dns = dns.astype(_BF16)
    xn_img = np.zeros((B, 256, H), dtype=_BF16)
    xn_img[:, :R, :] = img.astype(_BF16)

    in_maps = []
    for k in range(NCORES):
        sl = slice(k * BLOC, (k + 1) * BLOC)
        in_maps.append({
            "x8_dns": np.ascontiguousarray(x8_dns[sl]),
            "x8_img": np.ascontiguousarray(x8_img[sl]),
            "xb_dns": np.ascontiguousarray(xb_dns[sl]),
            "xb_img": np.ascontiguousarray(xb_img[sl]),
            "xn_dns": np.ascontiguousarray(xn_dns[sl]),
            "xn_img": np.ascontiguousarray(xn_img[sl]),
            "w8_i1": w8_i1, "wb_i1": wb_i1,
            "w8_d2": w8_d2, "wb_d2": wb_d2,
            "wrow_b": wrow_b, "wrow_d": wrow_d,
        })
    return in_maps


def kernel(**inputs):
    from concourse.bass_utils import run_bass_kernel_spmd

    nc = _get_nc()
    in_maps = make_in_maps(inputs)
    res = run_bass_kernel_spmd(nc, in_maps, list(range(NCORES))).results
    rows = np.concatenate([res[k]["out_rows"] for k in range(NCORES)], axis=0)
    att_img = np.ascontiguousarray(
        np.broadcast_to(rows[:, 0][:, None, :], (B, S, H)))
    att_dns = np.ascontiguousarray(
        np.broadcast_to(rows[:, 1][:, None, :], (B, S, H)))
    return att_dns, att_img


# revision 8
# speedup vs baseline: 1.2599x; 1.0106x over previous
"""CoAttention ImageDNS kernel for Trainium2 (8 NeuronCores, Bass/Tile).

Math: the reference computes two additive-attention blocks. In both, the
softmax'd score is  score[b, q, k] = f(q-side)[b, q] + g(k-side)[b, k] + c,
and softmax over k is invariant to the q-dependent (and constant) terms, so
the attention weights are independent of the query index:

  visual_att[b, s, :]  = softmax_r( wB . tanh(W_i1 @ img[b, r]) )
  textual_att[b, i, :] = softmax_j( wD . tanh(W_d2 @ dns[b, j]) )

Hence both outputs are per-batch rank-1 broadcasts:

  att_img_features[b, s, :] = visual_att[b]  @ img[b]   (same for all s)
  att_dns_features[b, i, :] = textual_att[b] @ dns[b]   (same for all i)

W_d1/b_d1/w_att1[:H]/b_att1/W_i2/b_i2/w_att2[:H]/b_att2 cancel entirely.

Sharding: pure data-parallel over batch, 4 batches per core, no collectives.

Perf notes vs the bf16 baseline (140.2us):
- Projection h-blocks 0..3 run as fp8(e4m3) DoubleRow matmuls with a REAL
  256-deep contraction per matmul (2 h-blocks per pair-column), which the PE
  streams at the same column rate as a 128-deep bf16 matmul -> 2x throughput
  on that half. Blocks 4..7 stay bf16. Net projection cost 0.75x, end-to-end
  rel err ~1.77e-2 vs the 2e-2 gate (fp8 on ALL blocks would be 2.6e-2; W is
  pre-scaled by 64 so its entries clear e4m3's subnormal floor).
- The tiny score-sum and stage-2 matmuls for item k are emitted AFTER all of
  item k+1's projection matmuls, so the PE stream never waits on the
  scalar/vector tanh/score chain (PE idle gaps re-throttle the HAM clock
  gate from 2.4GHz to 1.2GHz, which is what capped the baseline).
- Inputs stream on three DMA queues (weights on GpSimd's, x^T tiles on
  Sync's, stage-2 activations on Activation's) instead of one.
- Only one [1, H] output row per (batch, side) leaves the device; the
  broadcast over S is done on host (kills 16MB/core of output DMA).
- Stage 2 (attention-weighted sum of rows) stays bf16: e4m3 there would put
  ~3.6% error directly on the output.
"""

import sys
import numpy as np
import ml_dtypes

_BF16 = ml_dtypes.bfloat16
_E4M3 = ml_dtypes.float8_e4m3

for _p in ("/opt/trn_rl_repo", "/root/.axon_site/_ro/trn_rl_repo"):
    if _p not in sys.path:
        sys.path.append(_p)

B, S, R, H = 32, 512, 196, 1024
NCORES = 8
BLOC = B // NCORES          # batches per core
OC = 512                    # output-chunk (one fp32 PSUM bank)
NB8 = 4                     # h-blocks 0..3 in e4m3 (2 DoubleRow matmuls)
NBB = 4                     # h-blocks 4..7 in bf16
WSCALE = 64.0               # W pre-scale so e4m3 entries are normal numbers

_CACHE = {}


def _row_chunks(n):
    out, o = [], 0
    while o < n:
        out.append((o, min(128, n - o)))
        o += 128
    return out


def build_nc():
    from concourse import bacc, mybir
    from concourse import tile

    f32, f16, f8 = mybir.dt.float32, mybir.dt.bfloat16, mybir.dt.float8e4
    Act = mybir.ActivationFunctionType
    Alu = mybir.AluOpType
    DR = mybir.MatmulPerfMode.DoubleRow

    nc = bacc.Bacc("TRN2", target_bir_lowering=False, debug=False)

    RP = 256  # img row count padded to a partition multiple for single-DMA loads
    R8 = 208  # img rows padded so the DoubleRow pair-dim step is 16B-aligned
    x8_dns = nc.dram_tensor("x8_dns", [BLOC, 128, NB8 * S], f8, kind="ExternalInput")
    x8_img = nc.dram_tensor("x8_img", [BLOC, 128, NB8 * R8], f8, kind="ExternalInput")
    xb_dns = nc.dram_tensor("xb_dns", [BLOC, 128, NBB * S], f16, kind="ExternalInput")
    xb_img = nc.dram_tensor("xb_img", [BLOC, 128, NBB * R], f16, kind="ExternalInput")
    xn_dns = nc.dram_tensor("xn_dns", [BLOC, S, H], f16, kind="ExternalInput")
    xn_img = nc.dram_tensor("xn_img", [BLOC, RP, H], f16, kind="ExternalInput")
    w8_i1 = nc.dram_tensor("w8_i1", [NB8, 128, H], f8, kind="ExternalInput")
    wb_i1 = nc.dram_tensor("wb_i1", [NBB, 128, H], f16, kind="ExternalInput")
    w8_d2 = nc.dram_tensor("w8_d2", [NB8, 128, H], f8, kind="ExternalInput")
    wb_d2 = nc.dram_tensor("wb_d2", [NBB, 128, H], f16, kind="ExternalInput")
    wrow_b = nc.dram_tensor("wrow_b", [128, H], f32, kind="ExternalInput")
    wrow_d = nc.dram_tensor("wrow_d", [128, H], f32, kind="ExternalInput")
    out_rows = nc.dram_tensor("out_rows", [BLOC, 2, H], f32, kind="ExternalOutput")

    with tile.TileContext(nc) as tc:
        with (
            tc.tile_pool(name="const", bufs=1) as cpool,
            tc.tile_pool(name="xts", bufs=2) as xtpool,
            tc.tile_pool(name="xns", bufs=2) as xnpool,
            tc.tile_pool(name="work", bufs=3) as wpool,
            tc.tile_pool(name="small", bufs=2) as spool,
            tc.tile_pool(name="outs", bufs=2) as opool,
            tc.tile_pool(name="pp", bufs=2, space="PSUM") as ppool,
            tc.tile_pool(name="pa", bufs=2, space="PSUM") as papool,
            tc.tile_pool(name="ps", bufs=2, space="PSUM") as pstat,
        ):
            wt_sb, wrow_sb = {}, {}

            def get_wrow(nm):
                if nm not in wrow_sb:
                    dram = {"b": wrow_b, "d": wrow_d}[nm]
                    w = cpool.tile([128, H], f32, name=f"wrow_{nm}_sb")
                    nc.sync.dma_start(out=w[:, :], in_=dram[:, :])
                    wrow_sb[nm] = w
                return wrow_sb[nm]

            ones_col = cpool.tile([128, 1], f16, name="ones_col")
            nc.vector.memset(ones_col[:, :], 1.0)

            def emit_proj(b, side):
                n_rows = R if side == "img" else S
                n8 = R8 if side == "img" else S
                x8_d = x8_img if side == "img" else x8_dns
                xb_d = xb_img if side == "img" else xb_dns
                xn_d = xn_img if side == "img" else xn_dns
                wt_name = "i1" if side == "img" else "d2"
                # batch-0 tiles + first-use weights ride the Sync queue (the
                # first to start) in exact consumption order; later batches'
                # x tiles move to GpSimd's queue so they prefetch in parallel
                xq = nc.sync if b == 0 else nc.gpsimd
                load_wt = wt_name not in wt_sb
                if load_wt:
                    w8_d, wb_d = (w8_i1, wb_i1) if side == "img" else (w8_d2, wb_d2)
                    w8 = cpool.tile([128, NB8 * H], f8, name=f"w8_{wt_name}_sb")
                    wb = cpool.tile([128, NBB * H], f16, name=f"wb_{wt_name}_sb")
                    for j in range(NB8):
                        nc.sync.dma_start(out=w8[:, j * H:(j + 1) * H],
                                          in_=w8_d[j])
                    wt_sb[wt_name] = (w8, wb)
                w8, wb = wt_sb[wt_name]
                w8v = w8.rearrange("p (j o) -> p j o", j=NB8)
                rcs = _row_chunks(n_rows)

                x8_t = xtpool.tile([128, NB8 * n8], f8,
                                   name=f"x8_{side}_{b}", tag=f"x8_{side}")
                nc.sync.dma_start(out=x8_t[:, :], in_=x8_d[b]) if b == 0 else                     xq.dma_start(out=x8_t[:, :], in_=x8_d[b])
                if load_wt:
                    for j in range(NBB):
                        nc.sync.dma_start(out=wb[:, j * H:(j + 1) * H],
                                          in_=wb_d[j])
                xb_t = xtpool.tile([128, NBB * n_rows], f16,
                                   name=f"xb_{side}_{b}", tag=f"xb_{side}")
                xq.dma_start(out=xb_t[:, :], in_=xb_d[b])
                x8v = x8_t.rearrange("p (j m) -> p j m", j=NB8)  # j-stride n8

                acols = []
                xn_ts = []
                wr = None
                for ci, (r0, rk) in enumerate(rcs):
                    ps = ppool.tile([128, H], f32, name=f"proj_{side}_{ci}_{b}",
                                    tag="pp")
                    for u in range(NB8 // 2):
                        lhs = x8v[:, 2 * u:2 * u + 2, r0:r0 + rk]
                        for oc in range(2):
                            nc.tensor.matmul(
                                ps[0:rk, oc * OC:(oc + 1) * OC],
                                lhsT=lhs,
                                rhs=w8v[:, 2 * u:2 * u + 2, oc * OC:(oc + 1) * OC],
                                start=(u == 0), stop=False,
                                perf_mode=DR)
                    for j in range(NBB):
                        lhs = xb_t[:, j * n_rows + r0: j * n_rows + r0 + rk]
                        for oc in range(2):
                            nc.tensor.matmul(
                                ps[0:rk, oc * OC:(oc + 1) * OC],
                                lhsT=lhs,
                                rhs=wb[:, j * H + oc * OC: j * H + (oc + 1) * OC],
                                start=False, stop=(j == NBB - 1))
                    if ci == 0:
                        nrc = len(rcs)
                        xn_t = xnpool.tile([128, nrc * H], f16,
                                           name=f"xn_{side}_{b}", tag=f"xn_{side}")
                        # stage-2 activations stream on the Activation queue;
                        # they are consumed one pipeline item later
                        nc.scalar.dma_start(
                            out=xn_t.rearrange("p (rc n) -> p rc n", rc=nrc),
                            in_=xn_d[b, 0:nrc * 128, :]
                            .rearrange("(rc p) n -> p rc n", p=128))
                        xn_ts = [xn_t[:, cj * H:(cj + 1) * H] for cj in range(nrc)]
                        wr = get_wrow("b" if side == "img" else "d")
                    th = wpool.tile([128, H], f32, name=f"th_{side}_{ci}_{b}", tag="th")
                    nc.scalar.activation(th[0:rk, :], ps[0:rk, :], Act.Tanh,
                                         scale=1.0 / WSCALE)
                    scr = wpool.tile([128, H], f32, name=f"scr_{side}_{ci}_{b}",
                                     tag="scr", bufs=2)
                    tcol = spool.tile([128, 1], f32, name=f"tc_{side}_{ci}_{b}",
                                      tag="tcol", bufs=3)
                    nc.vector.scalar_tensor_tensor(
                        out=scr[0:rk, :], in0=th[0:rk, :], scalar=1.0,
                        in1=wr[0:rk, :], op0=Alu.mult, op1=Alu.mult,
                        accum_out=tcol[0:rk, :])
                    acol = spool.tile([128, 1], f16, name=f"a_{side}_{ci}_{b}",
                                      tag=f"acol_{side}_{ci}", bufs=2)
                    nc.scalar.activation(acol[0:rk, :], tcol[0:rk, :], Act.Exp)
                    acols.append((acol, rk))
                return (b, side, acols, xn_ts)

            def emit_reduce(state):
                b, side, acols, xn_ts = state
                sd = 0 if side == "img" else 1
                s_ps = pstat.tile([1, 1], f32, name=f"s_{side}_{b}", tag="stat")
                for ci, (acol, rk) in enumerate(acols):
                    nc.tensor.matmul(
                        s_ps[0:1, 0:1], lhsT=acol[0:rk, 0:1],
                        rhs=ones_col[0:rk, 0:1],
                        start=(ci == 0), stop=(ci == len(acols) - 1))
                r_sb = spool.tile([1, 1], f32, name=f"r_{side}_{b}", tag="r", bufs=2)
                nc.vector.reciprocal(r_sb[0:1, 0:1], s_ps[0:1, 0:1])
                att_sb = opool.tile([1, H], f32, name=f"attsb_{side}_{b}",
                                    tag="att")
                for oc in range(2):
                    att_ps = papool.tile([1, OC], f32,
                                         name=f"att_{side}_{b}_{oc}", tag="attps")
                    for ci, (acol, rk) in enumerate(acols):
                        nc.tensor.matmul(
                            att_ps[0:1, :],
                            lhsT=acol[0:rk, 0:1],
                            rhs=xn_ts[ci][0:rk, oc * OC:(oc + 1) * OC],
                            start=(ci == 0), stop=(ci == len(acols) - 1))
                    nc.scalar.activation(att_sb[0:1, oc * OC:(oc + 1) * OC],
                                         att_ps[0:1, :],
                                         Act.Copy, scale=r_sb[0:1, 0:1])
                nc.sync.dma_start(out=out_rows[b, sd:sd + 1, :],
                                  in_=att_sb[0:1, :])

            pending = None
            for b in range(BLOC):
                for side in ("img", "dns"):
                    state = emit_proj(b, side)
                    if pending is not None:
                        emit_reduce(pending)
                    pending = state
            emit_reduce(pending)
    nc.compile()
    return nc


def _get_nc():
    if "nc" not in _CACHE:
        _CACHE["nc"] = build_nc()
    return _CACHE["nc"]


def make_in_maps(inputs):
    dns = np.ascontiguousarray(np.asarray(inputs["dns_feature"], dtype=np.float32))
    img = np.ascontiguousarray(np.asarray(inputs["img_features"], dtype=np.float32))
    W_i1 = np.asarray(inputs["W_i1"], dtype=np.float32)
    W_d2 = np.asarray(inputs["W_d2"], dtype=np.float32)
    wB = np.asarray(inputs["w_att1"], dtype=np.float32)[H:]
    wD = np.asarray(inputs["w_att2"], dtype=np.float32)[H:]

    def pack_w(W):
        wt = np.ascontiguousarray(W.T) * WSCALE         # [h_in, o]
        w8 = np.ascontiguousarray(wt[:NB8 * 128].reshape(NB8, 128, H)).astype(_E4M3)
        wb = np.ascontiguousarray(wt[NB8 * 128:].reshape(NBB, 128, H)).astype(_BF16)
        return w8, wb
    w8_i1, wb_i1 = pack_w(W_i1)
    w8_d2, wb_d2 = pack_w(W_d2)
    wrow_b = np.ascontiguousarray(np.broadcast_to(wB, (128, H)))
    wrow_d = np.ascontiguousarray(np.broadcast_to(wD, (128, H)))

    def pack_x(x, n, n8):
        xt = x.transpose(0, 2, 1).reshape(B, 8, 128, n)
        x8 = np.zeros((B, NB8, 128, n8), dtype=_E4M3)
        x8[:, :, :, :n] = xt[:, :NB8].astype(_E4M3)
        x8 = np.ascontiguousarray(x8.transpose(0, 2, 1, 3).reshape(B, 128, NB8 * n8))
        xb = xt[:, NB8:].astype(_BF16)
        xb = np.ascontiguousarray(xb.transpose(0, 2, 1, 3).reshape(B, 128, NBB * n))
        return x8, xb
    x8_dns, xb_dns = pack_x(dns, S, S)
    x8_img, xb_img = pack_x(img, R, 208)
    xn_dns = dns.astype(_BF16)
    xn_img = np.zeros((B, 256, H), dtype=_BF16)
    xn_img[:, :R, :] = img.astype(_BF16)

    in_maps = []
    for k in range(NCORES):
        sl = slice(k * BLOC, (k + 1) * BLOC)
        in_maps.append({
            "x8_dns": np.ascontiguousarray(x8_dns[sl]),
            "x8_img": np.ascontiguousarray(x8_img[sl]),
            "xb_dns": np.ascontiguousarray(xb_dns[sl]),
            "xb_img": np.ascontiguousarray(xb_img[sl]),
            "xn_dns": np.ascontiguousarray(xn_dns[sl]),
            "xn_img": np.ascontiguousarray(xn_img[sl]),
            "w8_i1": w8_i1, "wb_i1": wb_i1,
            "w8_d2": w8_d2, "wb_d2": wb_d2,
            "wrow_b": wrow_b, "wrow_d": wrow_d,
        })
    return in_maps


def kernel(**inputs):
    from concourse.bass_utils import run_bass_kernel_spmd

    nc = _get_nc()
    in_maps = make_in_maps(inputs)
    res = run_bass_kernel_spmd(nc, in_maps, list(range(NCORES))).results
    rows = np.concatenate([res[k]["out_rows"] for k in range(NCORES)], axis=0)
    att_img = np.ascontiguousarray(
        np.broadcast_to(rows[:, 0][:, None, :], (B, S, H)))
    att_dns = np.ascontiguousarray(
        np.broadcast_to(rows[:, 1][:, None, :], (B, S, H)))
    return att_dns, att_img


# revision 9
# speedup vs baseline: 1.3195x; 1.0473x over previous
"""CoAttention ImageDNS kernel for Trainium2 (8 NeuronCores, Bass/Tile).

Math: the reference computes two additive-attention blocks. In both, the
softmax'd score is  score[b, q, k] = f(q-side)[b, q] + g(k-side)[b, k] + c,
and softmax over k is invariant to the q-dependent (and constant) terms, so
the attention weights are independent of the query index:

  visual_att[b, s, :]  = softmax_r( wB . tanh(W_i1 @ img[b, r]) )
  textual_att[b, i, :] = softmax_j( wD . tanh(W_d2 @ dns[b, j]) )

Hence both outputs are per-batch rank-1 broadcasts:

  att_img_features[b, s, :] = visual_att[b]  @ img[b]   (same for all s)
  att_dns_features[b, i, :] = textual_att[b] @ dns[b]   (same for all i)

W_d1/b_d1/w_att1[:H]/b_att1/W_i2/b_i2/w_att2[:H]/b_att2 cancel entirely.

Sharding: pure data-parallel over batch, 4 batches per core, no collectives.

Perf notes vs the bf16 baseline (140.2us):
- Projection h-blocks 0..3 run as fp8(e4m3) DoubleRow matmuls with a REAL
  256-deep contraction per matmul (2 h-blocks per pair-column), which the PE
  streams at the same column rate as a 128-deep bf16 matmul -> 2x throughput
  on that half. Blocks 4..7 stay bf16. Net projection cost 0.75x, end-to-end
  rel err ~1.77e-2 vs the 2e-2 gate (fp8 on ALL blocks would be 2.6e-2; W is
  pre-scaled by 64 so its entries clear e4m3's subnormal floor).
- The tiny score-sum and stage-2 matmuls for item k are emitted AFTER all of
  item k+1's projection matmuls, so the PE stream never waits on the
  scalar/vector tanh/score chain (PE idle gaps re-throttle the HAM clock
  gate from 2.4GHz to 1.2GHz, which is what capped the baseline).
- Inputs stream on three DMA queues (weights on GpSimd's, x^T tiles on
  Sync's, stage-2 activations on Activation's) instead of one.
- Only one [1, H] output row per (batch, side) leaves the device; the
  broadcast over S is done on host (kills 16MB/core of output DMA).
- Stage 2 (attention-weighted sum of rows) stays bf16: e4m3 there would put
  ~3.6% error directly on the output.
"""

import sys
import numpy as np
import ml_dtypes

_BF16 = ml_dtypes.bfloat16
_E4M3 = ml_dtypes.float8_e4m3

for _p in ("/opt/trn_rl_repo", "/root/.axon_site/_ro/trn_rl_repo"):
    if _p not in sys.path:
        sys.path.append(_p)

B, S, R, H = 32, 512, 196, 1024
NCORES = 8
BLOC = B // NCORES          # batches per core
OC = 512                    # output-chunk (one fp32 PSUM bank)
NB8 = 4                     # h-blocks 0..3 in e4m3 (2 DoubleRow matmuls)
NBB = 4                     # h-blocks 4..7 in bf16
WSCALE = 64.0               # W pre-scale so e4m3 entries are normal numbers

_CACHE = {}


def _row_chunks(n):
    out, o = [], 0
    while o < n:
        out.append((o, min(128, n - o)))
        o += 128
    return out


def build_nc():
    from concourse import bacc, mybir
    from concourse import tile

    f32, f16, f8 = mybir.dt.float32, mybir.dt.bfloat16, mybir.dt.float8e4
    Act = mybir.ActivationFunctionType
    Alu = mybir.AluOpType
    DR = mybir.MatmulPerfMode.DoubleRow

    nc = bacc.Bacc("TRN2", target_bir_lowering=False, debug=False)

    RP = 256  # img row count padded to a partition multiple for single-DMA loads
    R8 = 208  # img rows padded so the DoubleRow pair-dim step is 16B-aligned
    x8_dns = nc.dram_tensor("x8_dns", [BLOC, 128, NB8 * S], f8, kind="ExternalInput")
    x8_img = nc.dram_tensor("x8_img", [BLOC, 128, NB8 * R8], f8, kind="ExternalInput")
    xb_dns = nc.dram_tensor("xb_dns", [BLOC, 128, NBB * S], f16, kind="ExternalInput")
    xb_img = nc.dram_tensor("xb_img", [BLOC, 128, NBB * R], f16, kind="ExternalInput")
    xn_dns = nc.dram_tensor("xn_dns", [BLOC, 128, 4 * H], f16, kind="ExternalInput")
    xn_img = nc.dram_tensor("xn_img", [BLOC, 128, 2 * H], f16, kind="ExternalInput")
    w8_i1 = nc.dram_tensor("w8_i1", [128, NB8 * H], f8, kind="ExternalInput")
    wb_i1 = nc.dram_tensor("wb_i1", [128, NBB * H], f16, kind="ExternalInput")
    w8_d2 = nc.dram_tensor("w8_d2", [128, NB8 * H], f8, kind="ExternalInput")
    wb_d2 = nc.dram_tensor("wb_d2", [128, NBB * H], f16, kind="ExternalInput")
    wrow_b = nc.dram_tensor("wrow_b", [128, H], f32, kind="ExternalInput")
    wrow_d = nc.dram_tensor("wrow_d", [128, H], f32, kind="ExternalInput")
    out_rows = nc.dram_tensor("out_rows", [BLOC, 2, H], f32, kind="ExternalOutput")

    with tile.TileContext(nc) as tc:
        with (
            tc.tile_pool(name="const", bufs=1) as cpool,
            tc.tile_pool(name="xts", bufs=2) as xtpool,
            tc.tile_pool(name="xns", bufs=2) as xnpool,
            tc.tile_pool(name="work", bufs=3) as wpool,
            tc.tile_pool(name="small", bufs=2) as spool,
            tc.tile_pool(name="outs", bufs=2) as opool,
            tc.tile_pool(name="pp", bufs=2, space="PSUM") as ppool,
            tc.tile_pool(name="pa", bufs=2, space="PSUM") as papool,
            tc.tile_pool(name="ps", bufs=2, space="PSUM") as pstat,
        ):
            wt_sb, wrow_sb = {}, {}

            def get_wrow(nm):
                if nm not in wrow_sb:
                    dram = {"b": wrow_b, "d": wrow_d}[nm]
                    w = cpool.tile([128, H], f32, name=f"wrow_{nm}_sb")
                    nc.scalar.dma_start(out=w[:, :], in_=dram[:, :])
                    wrow_sb[nm] = w
                return wrow_sb[nm]

            ones_col = cpool.tile([128, 1], f16, name="ones_col")
            nc.vector.memset(ones_col[:, :], 1.0)

            def emit_proj(b, side):
                n_rows = R if side == "img" else S
                n8 = R8 if side == "img" else S
                x8_d = x8_img if side == "img" else x8_dns
                xb_d = xb_img if side == "img" else xb_dns
                xn_d = xn_img if side == "img" else xn_dns
                wt_name = "i1" if side == "img" else "d2"
                # batch-0 tiles + first-use weights ride the Sync queue (the
                # first to start) in exact consumption order; later batches'
                # x tiles move to GpSimd's queue so they prefetch in parallel
                xq = nc.sync if b == 0 else nc.gpsimd
                load_wt = wt_name not in wt_sb
                if load_wt:
                    w8_d, wb_d = (w8_i1, wb_i1) if side == "img" else (w8_d2, wb_d2)
                    w8 = cpool.tile([128, NB8 * H], f8, name=f"w8_{wt_name}_sb")
                    wb = cpool.tile([128, NBB * H], f16, name=f"wb_{wt_name}_sb")
                    nc.sync.dma_start(out=w8[:, :], in_=w8_d[:, :])
                    wt_sb[wt_name] = (w8, wb)
                w8, wb = wt_sb[wt_name]
                w8v = w8.rearrange("p (j o) -> p j o", j=NB8)
                rcs = _row_chunks(n_rows)

                x8_t = xtpool.tile([128, NB8 * n8], f8,
                                   name=f"x8_{side}_{b}", tag=f"x8_{side}")
                nc.sync.dma_start(out=x8_t[:, :], in_=x8_d[b]) if b == 0 else                     xq.dma_start(out=x8_t[:, :], in_=x8_d[b])
                if load_wt:
                    nc.sync.dma_start(out=wb[:, :], in_=wb_d[:, :])
                xb_t = xtpool.tile([128, NBB * n_rows], f16,
                                   name=f"xb_{side}_{b}", tag=f"xb_{side}")
                xq.dma_start(out=xb_t[:, :], in_=xb_d[b])
                x8v = x8_t.rearrange("p (j m) -> p j m", j=NB8)  # j-stride n8

                acols = []
                xn_ts = []
                wr = None
                # chunks are emitted in pairs with the fp8 groups of both
                # chunks back-to-back, halving PE fp8<->bf16 mode switches
                def emit_dr(ci, r0, rk):
                    ps = ppool.tile([128, H], f32, name=f"proj_{side}_{ci}_{b}",
                                    tag="pp")
                    for u in range(NB8 // 2):
                        lhs = x8v[:, 2 * u:2 * u + 2, r0:r0 + rk]
                        for oc in range(2):
                            nc.tensor.matmul(
                                ps[0:rk, oc * OC:(oc + 1) * OC],
                                lhsT=lhs,
                                rhs=w8v[:, 2 * u:2 * u + 2, oc * OC:(oc + 1) * OC],
                                start=(u == 0), stop=False,
                                perf_mode=DR)
                    return ps

                def emit_bf(ci, r0, rk, ps):
                    for j in range(NBB):
                        lhs = xb_t[:, j * n_rows + r0: j * n_rows + r0 + rk]
                        for oc in range(2):
                            nc.tensor.matmul(
                                ps[0:rk, oc * OC:(oc + 1) * OC],
                                lhsT=lhs,
                                rhs=wb[:, j * H + oc * OC: j * H + (oc + 1) * OC],
                                start=False, stop=(j == NBB - 1))

                def emit_act(ci, r0, rk, ps):
                    th = wpool.tile([128, H], f32, name=f"th_{side}_{ci}_{b}", tag="th")
                    nc.scalar.activation(th[0:rk, :], ps[0:rk, :], Act.Tanh,
                                         scale=1.0 / WSCALE)
                    scr = wpool.tile([128, H], f32, name=f"scr_{side}_{ci}_{b}",
                                     tag="scr", bufs=2)
                    tcol = spool.tile([128, 1], f32, name=f"tc_{side}_{ci}_{b}",
                                      tag="tcol", bufs=3)
                    nc.vector.scalar_tensor_tensor(
                        out=scr[0:rk, :], in0=th[0:rk, :], scalar=1.0,
                        in1=wr[0:rk, :], op0=Alu.mult, op1=Alu.mult,
                        accum_out=tcol[0:rk, :])
                    acol = spool.tile([128, 1], f16, name=f"a_{side}_{ci}_{b}",
                                      tag=f"acol_{side}_{ci}", bufs=2)
                    nc.scalar.activation(acol[0:rk, :], tcol[0:rk, :], Act.Exp)
                    acols.append((acol, rk))

                for c0 in range(0, len(rcs), 2):
                    pair = [(ci, rcs[ci]) for ci in range(c0, min(c0 + 2, len(rcs)))]
                    pss = [emit_dr(ci, r0, rk) for ci, (r0, rk) in pair]
                    if c0 == 0:
                        nrc = len(rcs)
                        xn_t = xnpool.tile([128, nrc * H], f16,
                                           name=f"xn_{side}_{b}", tag=f"xn_{side}")
                        # stage-2 activations stream on the Activation queue;
                        # they are consumed one pipeline item later
                        nc.scalar.dma_start(out=xn_t[:, :], in_=xn_d[b])
                        xn_ts = [xn_t[:, cj * H:(cj + 1) * H] for cj in range(nrc)]
                        wr = get_wrow("b" if side == "img" else "d")
                    for (ci, (r0, rk)), ps in zip(pair, pss):
                        emit_bf(ci, r0, rk, ps)
                    for (ci, (r0, rk)), ps in zip(pair, pss):
                        emit_act(ci, r0, rk, ps)
                return (b, side, acols, xn_ts)

            def emit_reduce(state):
                b, side, acols, xn_ts = state
                sd = 0 if side == "img" else 1
                s_ps = pstat.tile([1, 1], f32, name=f"s_{side}_{b}", tag="stat")
                for ci, (acol, rk) in enumerate(acols):
                    nc.tensor.matmul(
                        s_ps[0:1, 0:1], lhsT=acol[0:rk, 0:1],
                        rhs=ones_col[0:rk, 0:1],
                        start=(ci == 0), stop=(ci == len(acols) - 1))
                r_sb = spool.tile([1, 1], f32, name=f"r_{side}_{b}", tag="r", bufs=2)
                nc.vector.reciprocal(r_sb[0:1, 0:1], s_ps[0:1, 0:1])
                att_sb = opool.tile([1, H], f32, name=f"attsb_{side}_{b}",
                                    tag="att")
                for oc in range(2):
                    att_ps = papool.tile([1, OC], f32,
                                         name=f"att_{side}_{b}_{oc}", tag="attps")
                    for ci, (acol, rk) in enumerate(acols):
                        nc.tensor.matmul(
                            att_ps[0:1, :],
                            lhsT=acol[0:rk, 0:1],
                            rhs=xn_ts[ci][0:rk, oc * OC:(oc + 1) * OC],
                            start=(ci == 0), stop=(ci == len(acols) - 1))
                    nc.scalar.activation(att_sb[0:1, oc * OC:(oc + 1) * OC],
                                         att_ps[0:1, :],
                                         Act.Copy, scale=r_sb[0:1, 0:1])
                nc.sync.dma_start(out=out_rows[b, sd:sd + 1, :],
                                  in_=att_sb[0:1, :])

            pending = None
            for b in range(BLOC):
                for side in ("img", "dns"):
                    state = emit_proj(b, side)
                    if pending is not None:
                        emit_reduce(pending)
                    pending = state
            emit_reduce(pending)
    nc.compile()
    return nc


def _get_nc():
    if "nc" not in _CACHE:
        _CACHE["nc"] = build_nc()
    return _CACHE["nc"]


def make_in_maps(inputs):
    dns = np.ascontiguousarray(np.asarray(inputs["dns_feature"], dtype=np.float32))
    img = np.ascontiguousarray(np.asarray(inputs["img_features"], dtype=np.float32))
    W_i1 = np.asarray(inputs["W_i1"], dtype=np.float32)
    W_d2 = np.asarray(inputs["W_d2"], dtype=np.float32)
    wB = np.asarray(inputs["w_att1"], dtype=np.float32)[H:]
    wD = np.asarray(inputs["w_att2"], dtype=np.float32)[H:]

    def pack_w(W):
        wt = np.ascontiguousarray(W.T) * WSCALE         # [h_in, o]
        w8 = np.ascontiguousarray(
            wt[:NB8 * 128].reshape(NB8, 128, H).transpose(1, 0, 2)
            .reshape(128, NB8 * H)).astype(_E4M3)
        wb = np.ascontiguousarray(
            wt[NB8 * 128:].reshape(NBB, 128, H).transpose(1, 0, 2)
            .reshape(128, NBB * H)).astype(_BF16)
        return w8, wb
    w8_i1, wb_i1 = pack_w(W_i1)
    w8_d2, wb_d2 = pack_w(W_d2)
    wrow_b = np.ascontiguousarray(np.broadcast_to(wB, (128, H)))
    wrow_d = np.ascontiguousarray(np.broadcast_to(wD, (128, H)))

    def pack_x(x, n, n8):
        xt = x.transpose(0, 2, 1).reshape(B, 8, 128, n)
        x8 = np.zeros((B, NB8, 128, n8), dtype=_E4M3)
        x8[:, :, :, :n] = xt[:, :NB8].astype(_E4M3)
        x8 = np.ascontiguousarray(x8.transpose(0, 2, 1, 3).reshape(B, 128, NB8 * n8))
        xb = xt[:, NB8:].astype(_BF16)
        xb = np.ascontiguousarray(xb.transpose(0, 2, 1, 3).reshape(B, 128, NBB * n))
        return x8, xb
    x8_dns, xb_dns = pack_x(dns, S, S)
    x8_img, xb_img = pack_x(img, R, 208)
    def pack_xn(x, nrc):
        xp = np.zeros((B, nrc * 128, H), dtype=np.float32)
        xp[:, :x.shape[1]] = x
        return np.ascontiguousarray(
            xp.reshape(B, nrc, 128, H).transpose(0, 2, 1, 3)
            .reshape(B, 128, nrc * H)).astype(_BF16)
    xn_dns = pack_xn(dns, 4)
    xn_img = pack_xn(img, 2)

    in_maps = []
    for k in range(NCORES):
        sl = slice(k * BLOC, (k + 1) * BLOC)
        in_maps.append({
            "x8_dns": np.ascontiguousarray(x8_dns[sl]),
            "x8_img": np.ascontiguousarray(x8_img[sl]),
            "xb_dns": np.ascontiguousarray(xb_dns[sl]),
            "xb_img": np.ascontiguousarray(xb_img[sl]),
            "xn_dns": np.ascontiguousarray(xn_dns[sl]),
            "xn_img": np.ascontiguousarray(xn_img[sl]),
            "w8_i1": w8_i1, "wb_i1": wb_i1,
            "w8_d2": w8_d2, "wb_d2": wb_d2,
            "wrow_b": wrow_b, "wrow_d": wrow_d,
        })
    return in_maps


def kernel(**inputs):
    from concourse.bass_utils import run_bass_kernel_spmd

    nc = _get_nc()
    in_maps = make_in_maps(inputs)
    res = run_bass_kernel_spmd(nc, in_maps, list(range(NCORES))).results
    rows = np.concatenate([res[k]["out_rows"] for k in range(NCORES)], axis=0)
    att_img = np.ascontiguousarray(
        np.broadcast_to(rows[:, 0][:, None, :], (B, S, H)))
    att_dns = np.ascontiguousarray(
        np.broadcast_to(rows[:, 1][:, None, :], (B, S, H)))
    return att_dns, att_img


# revision 10
# speedup vs baseline: 1.3212x; 1.0014x over previous
"""CoAttention ImageDNS kernel for Trainium2 (8 NeuronCores, Bass/Tile).

Math: the reference computes two additive-attention blocks. In both, the
softmax'd score is  score[b, q, k] = f(q-side)[b, q] + g(k-side)[b, k] + c,
and softmax over k is invariant to the q-dependent (and constant) terms, so
the attention weights are independent of the query index:

  visual_att[b, s, :]  = softmax_r( wB . tanh(W_i1 @ img[b, r]) )
  textual_att[b, i, :] = softmax_j( wD . tanh(W_d2 @ dns[b, j]) )

Hence both outputs are per-batch rank-1 broadcasts:

  att_img_features[b, s, :] = visual_att[b]  @ img[b]   (same for all s)
  att_dns_features[b, i, :] = textual_att[b] @ dns[b]   (same for all i)

W_d1/b_d1/w_att1[:H]/b_att1/W_i2/b_i2/w_att2[:H]/b_att2 cancel entirely.

Sharding: pure data-parallel over batch, 4 batches per core, no collectives.

Perf notes vs the bf16 baseline (140.2us):
- Projection h-blocks 0..3 run as fp8(e4m3) DoubleRow matmuls with a REAL
  256-deep contraction per matmul (2 h-blocks per pair-column), which the PE
  streams at the same column rate as a 128-deep bf16 matmul -> 2x throughput
  on that half. Blocks 4..7 stay bf16. Net projection cost 0.75x, end-to-end
  rel err ~1.77e-2 vs the 2e-2 gate (fp8 on ALL blocks would be 2.6e-2; W is
  pre-scaled by 64 so its entries clear e4m3's subnormal floor).
- The tiny score-sum and stage-2 matmuls for item k are emitted AFTER all of
  item k+1's projection matmuls, so the PE stream never waits on the
  scalar/vector tanh/score chain (PE idle gaps re-throttle the HAM clock
  gate from 2.4GHz to 1.2GHz, which is what capped the baseline).
- Inputs stream on three DMA queues (weights on GpSimd's, x^T tiles on
  Sync's, stage-2 activations on Activation's) instead of one.
- Only one [1, H] output row per (batch, side) leaves the device; the
  broadcast over S is done on host (kills 16MB/core of output DMA).
- Stage 2 (attention-weighted sum of rows) stays bf16: e4m3 there would put
  ~3.6% error directly on the output.
"""

import sys
import numpy as np
import ml_dtypes

_BF16 = ml_dtypes.bfloat16
_E4M3 = ml_dtypes.float8_e4m3

for _p in ("/opt/trn_rl_repo", "/root/.axon_site/_ro/trn_rl_repo"):
    if _p not in sys.path:
        sys.path.append(_p)

B, S, R, H = 32, 512, 196, 1024
NCORES = 8
BLOC = B // NCORES          # batches per core
OC = 512                    # output-chunk (one fp32 PSUM bank)
NB8 = 4                     # h-blocks 0..3 in e4m3 (2 DoubleRow matmuls)
NBB = 4                     # h-blocks 4..7 in bf16
WSCALE = 64.0               # W pre-scale so e4m3 entries are normal numbers

_CACHE = {}


def _row_chunks(n):
    out, o = [], 0
    while o < n:
        out.append((o, min(128, n - o)))
        o += 128
    return out


def build_nc():
    from concourse import bacc, mybir
    from concourse import tile

    f32, f16, f8 = mybir.dt.float32, mybir.dt.bfloat16, mybir.dt.float8e4
    Act = mybir.ActivationFunctionType
    Alu = mybir.AluOpType
    DR = mybir.MatmulPerfMode.DoubleRow

    nc = bacc.Bacc("TRN2", target_bir_lowering=False, debug=False)

    RP = 256  # img row count padded to a partition multiple for single-DMA loads
    R8 = 208  # img rows padded so the DoubleRow pair-dim step is 16B-aligned
    x8_dns = nc.dram_tensor("x8_dns", [BLOC, 128, NB8 * S], f8, kind="ExternalInput")
    x8_img = nc.dram_tensor("x8_img", [BLOC, 128, NB8 * R8], f8, kind="ExternalInput")
    xb_dns = nc.dram_tensor("xb_dns", [BLOC, 128, NBB * S], f16, kind="ExternalInput")
    xb_img = nc.dram_tensor("xb_img", [BLOC, 128, NBB * R], f16, kind="ExternalInput")
    xn_dns = nc.dram_tensor("xn_dns", [BLOC, 128, 4 * H], f16, kind="ExternalInput")
    xn_img = nc.dram_tensor("xn_img", [BLOC, 128, 2 * H], f16, kind="ExternalInput")
    w8_i1 = nc.dram_tensor("w8_i1", [128, NB8 * H], f8, kind="ExternalInput")
    wb_i1 = nc.dram_tensor("wb_i1", [128, NBB * H], f16, kind="ExternalInput")
    w8_d2 = nc.dram_tensor("w8_d2", [128, NB8 * H], f8, kind="ExternalInput")
    wb_d2 = nc.dram_tensor("wb_d2", [128, NBB * H], f16, kind="ExternalInput")
    wrow_b = nc.dram_tensor("wrow_b", [128, H], f32, kind="ExternalInput")
    wrow_d = nc.dram_tensor("wrow_d", [128, H], f32, kind="ExternalInput")
    out_rows = nc.dram_tensor("out_rows", [BLOC, 2, H], f32, kind="ExternalOutput")

    with tile.TileContext(nc) as tc:
        with (
            tc.tile_pool(name="const", bufs=1) as cpool,
            tc.tile_pool(name="xts", bufs=2) as xtpool,
            tc.tile_pool(name="xns", bufs=2) as xnpool,
            tc.tile_pool(name="work", bufs=3) as wpool,
            tc.tile_pool(name="small", bufs=2) as spool,
            tc.tile_pool(name="outs", bufs=2) as opool,
            tc.tile_pool(name="pp", bufs=2, space="PSUM") as ppool,
            tc.tile_pool(name="pa", bufs=2, space="PSUM") as papool,
            tc.tile_pool(name="ps", bufs=2, space="PSUM") as pstat,
        ):
            wt_sb, wrow_sb = {}, {}

            def get_wrow(nm):
                if nm not in wrow_sb:
                    dram = {"b": wrow_b, "d": wrow_d}[nm]
                    w = cpool.tile([128, H], f32, name=f"wrow_{nm}_sb")
                    nc.scalar.dma_start(out=w[:, :], in_=dram[:, :])
                    wrow_sb[nm] = w
                return wrow_sb[nm]

            ones_col = cpool.tile([128, 1], f16, name="ones_col")
            nc.vector.memset(ones_col[:, :], 1.0)

            def emit_proj(b, side):
                n_rows = R if side == "img" else S
                n8 = R8 if side == "img" else S
                x8_d = x8_img if side == "img" else x8_dns
                xb_d = xb_img if side == "img" else xb_dns
                xn_d = xn_img if side == "img" else xn_dns
                wt_name = "i1" if side == "img" else "d2"
                # batch-0 tiles + first-use weights ride the Sync queue (the
                # first to start) in exact consumption order; later batches'
                # x tiles move to GpSimd's queue so they prefetch in parallel
                xq = nc.sync if b == 0 else nc.gpsimd
                load_wt = wt_name not in wt_sb
                if load_wt:
                    w8_d, wb_d = (w8_i1, wb_i1) if side == "img" else (w8_d2, wb_d2)
                    w8 = cpool.tile([128, NB8 * H], f8, name=f"w8_{wt_name}_sb")
                    wb = cpool.tile([128, NBB * H], f16, name=f"wb_{wt_name}_sb")
                    for jh in range(2):
                        nc.sync.dma_start(
                            out=w8[:, jh * 2 * H:(jh + 1) * 2 * H],
                            in_=w8_d[:, jh * 2 * H:(jh + 1) * 2 * H])
                    wt_sb[wt_name] = (w8, wb)
                w8, wb = wt_sb[wt_name]
                w8v = w8.rearrange("p (j o) -> p j o", j=NB8)
                rcs = _row_chunks(n_rows)

                x8_t = xtpool.tile([128, NB8 * n8], f8,
                                   name=f"x8_{side}_{b}", tag=f"x8_{side}")
                nc.sync.dma_start(out=x8_t[:, :], in_=x8_d[b]) if b == 0 else                     xq.dma_start(out=x8_t[:, :], in_=x8_d[b])
                if load_wt:
                    for j in range(NBB):
                        nc.sync.dma_start(out=wb[:, j * H:(j + 1) * H],
                                          in_=wb_d[:, j * H:(j + 1) * H])
                xb_t = xtpool.tile([128, NBB * n_rows], f16,
                                   name=f"xb_{side}_{b}", tag=f"xb_{side}")
                xq.dma_start(out=xb_t[:, :], in_=xb_d[b])
                x8v = x8_t.rearrange("p (j m) -> p j m", j=NB8)  # j-stride n8

                acols = []
                xn_ts = []
                wr = None
                # chunks are emitted in pairs with the fp8 groups of both
                # chunks back-to-back, halving PE fp8<->bf16 mode switches
                def emit_dr(ci, r0, rk):
                    ps = ppool.tile([128, H], f32, name=f"proj_{side}_{ci}_{b}",
                                    tag="pp")
                    for u in range(NB8 // 2):
                        lhs = x8v[:, 2 * u:2 * u + 2, r0:r0 + rk]
                        for oc in range(2):
                            nc.tensor.matmul(
                                ps[0:rk, oc * OC:(oc + 1) * OC],
                                lhsT=lhs,
                                rhs=w8v[:, 2 * u:2 * u + 2, oc * OC:(oc + 1) * OC],
                                start=(u == 0), stop=False,
                                perf_mode=DR)
                    return ps

                def emit_bf(ci, r0, rk, ps):
                    for j in range(NBB):
                        lhs = xb_t[:, j * n_rows + r0: j * n_rows + r0 + rk]
                        for oc in range(2):
                            nc.tensor.matmul(
                                ps[0:rk, oc * OC:(oc + 1) * OC],
                                lhsT=lhs,
                                rhs=wb[:, j * H + oc * OC: j * H + (oc + 1) * OC],
                                start=False, stop=(j == NBB - 1))

                def emit_act(ci, r0, rk, ps):
                    th = wpool.tile([128, H], f32, name=f"th_{side}_{ci}_{b}", tag="th")
                    nc.scalar.activation(th[0:rk, :], ps[0:rk, :], Act.Tanh,
                                         scale=1.0 / WSCALE)
                    scr = wpool.tile([128, H], f32, name=f"scr_{side}_{ci}_{b}",
                                     tag="scr", bufs=2)
                    tcol = spool.tile([128, 1], f32, name=f"tc_{side}_{ci}_{b}",
                                      tag="tcol", bufs=3)
                    nc.vector.scalar_tensor_tensor(
                        out=scr[0:rk, :], in0=th[0:rk, :], scalar=1.0,
                        in1=wr[0:rk, :], op0=Alu.mult, op1=Alu.mult,
                        accum_out=tcol[0:rk, :])
                    acol = spool.tile([128, 1], f16, name=f"a_{side}_{ci}_{b}",
                                      tag=f"acol_{side}_{ci}", bufs=2)
                    nc.scalar.activation(acol[0:rk, :], tcol[0:rk, :], Act.Exp)
                    acols.append((acol, rk))

                for c0 in range(0, len(rcs), 2):
                    pair = [(ci, rcs[ci]) for ci in range(c0, min(c0 + 2, len(rcs)))]
                    pss = [emit_dr(ci, r0, rk) for ci, (r0, rk) in pair]
                    if c0 == 0:
                        nrc = len(rcs)
                        xn_t = xnpool.tile([128, nrc * H], f16,
                                           name=f"xn_{side}_{b}", tag=f"xn_{side}")
                        # stage-2 activations stream on the Activation queue;
                        # they are consumed one pipeline item later
                        nc.scalar.dma_start(out=xn_t[:, :], in_=xn_d[b])
                        xn_ts = [xn_t[:, cj * H:(cj + 1) * H] for cj in range(nrc)]
                        wr = get_wrow("b" if side == "img" else "d")
                    for (ci, (r0, rk)), ps in zip(pair, pss):
                        emit_bf(ci, r0, rk, ps)
                    for (ci, (r0, rk)), ps in zip(pair, pss):
                        emit_act(ci, r0, rk, ps)
                return (b, side, acols, xn_ts)

            def emit_reduce(state):
                b, side, acols, xn_ts = state
                sd = 0 if side == "img" else 1
                s_ps = pstat.tile([1, 1], f32, name=f"s_{side}_{b}", tag="stat")
                for ci, (acol, rk) in enumerate(acols):
                    nc.tensor.matmul(
                        s_ps[0:1, 0:1], lhsT=acol[0:rk, 0:1],
                        rhs=ones_col[0:rk, 0:1],
                        start=(ci == 0), stop=(ci == len(acols) - 1))
                r_sb = spool.tile([1, 1], f32, name=f"r_{side}_{b}", tag="r", bufs=2)
                nc.vector.reciprocal(r_sb[0:1, 0:1], s_ps[0:1, 0:1])
                att_sb = opool.tile([1, H], f32, name=f"attsb_{side}_{b}",
                                    tag="att")
                for oc in range(2):
                    att_ps = papool.tile([1, OC], f32,
                                         name=f"att_{side}_{b}_{oc}", tag="attps")
                    for ci, (acol, rk) in enumerate(acols):
                        nc.tensor.matmul(
                            att_ps[0:1, :],
                            lhsT=acol[0:rk, 0:1],
                            rhs=xn_ts[ci][0:rk, oc * OC:(oc + 1) * OC],
                            start=(ci == 0), stop=(ci == len(acols) - 1))
                    nc.scalar.activation(att_sb[0:1, oc * OC:(oc + 1) * OC],
                                         att_ps[0:1, :],
                                         Act.Copy, scale=r_sb[0:1, 0:1])
                nc.sync.dma_start(out=out_rows[b, sd:sd + 1, :],
                                  in_=att_sb[0:1, :])

            pending = None
            for b in range(BLOC):
                for side in ("img", "dns"):
                    state = emit_proj(b, side)
                    if pending is not None:
                        emit_reduce(pending)
                    pending = state
            emit_reduce(pending)
    nc.compile()
    return nc


def _get_nc():
    if "nc" not in _CACHE:
        _CACHE["nc"] = build_nc()
    return _CACHE["nc"]


def make_in_maps(inputs):
    dns = np.ascontiguousarray(np.asarray(inputs["dns_feature"], dtype=np.float32))
    img = np.ascontiguousarray(np.asarray(inputs["img_features"], dtype=np.float32))
    W_i1 = np.asarray(inputs["W_i1"], dtype=np.float32)
    W_d2 = np.asarray(inputs["W_d2"], dtype=np.float32)
    wB = np.asarray(inputs["w_att1"], dtype=np.float32)[H:]
    wD = np.asarray(inputs["w_att2"], dtype=np.float32)[H:]

    def pack_w(W):
        wt = np.ascontiguousarray(W.T) * WSCALE         # [h_in, o]
        w8 = np.ascontiguousarray(
            wt[:NB8 * 128].reshape(NB8, 128, H).transpose(1, 0, 2)
            .reshape(128, NB8 * H)).astype(_E4M3)
        wb = np.ascontiguousarray(
            wt[NB8 * 128:].reshape(NBB, 128, H).transpose(1, 0, 2)
            .reshape(128, NBB * H)).astype(_BF16)
        return w8, wb
    w8_i1, wb_i1 = pack_w(W_i1)
    w8_d2, wb_d2 = pack_w(W_d2)
    wrow_b = np.ascontiguousarray(np.broadcast_to(wB, (128, H)))
    wrow_d = np.ascontiguousarray(np.broadcast_to(wD, (128, H)))

    def pack_x(x, n, n8):
        xt = x.transpose(0, 2, 1).reshape(B, 8, 128, n)
        x8 = np.zeros((B, NB8, 128, n8), dtype=_E4M3)
        x8[:, :, :, :n] = xt[:, :NB8].astype(_E4M3)
        x8 = np.ascontiguousarray(x8.transpose(0, 2, 1, 3).reshape(B, 128, NB8 * n8))
        xb = xt[:, NB8:].astype(_BF16)
        xb = np.ascontiguousarray(xb.transpose(0, 2, 1, 3).reshape(B, 128, NBB * n))
        return x8, xb
    x8_dns, xb_dns = pack_x(dns, S, S)
    x8_img, xb_img = pack_x(img, R, 208)
    def pack_xn(x, nrc):
        xp = np.zeros((B, nrc * 128, H), dtype=np.float32)
        xp[:, :x.shape[1]] = x
        return np.ascontiguousarray(
            xp.reshape(B, nrc, 128, H).transpose(0, 2, 1, 3)
            .reshape(B, 128, nrc * H)).astype(_BF16)
    xn_dns = pack_xn(dns, 4)
    xn_img = pack_xn(img, 2)

    in_maps = []
    for k in range(NCORES):
        sl = slice(k * BLOC, (k + 1) * BLOC)
        in_maps.append({
            "x8_dns": np.ascontiguousarray(x8_dns[sl]),
            "x8_img": np.ascontiguousarray(x8_img[sl]),
            "xb_dns": np.ascontiguousarray(xb_dns[sl]),
            "xb_img": np.ascontiguousarray(xb_img[sl]),
            "xn_dns": np.ascontiguousarray(xn_dns[sl]),
            "xn_img": np.ascontiguousarray(xn_img[sl]),
            "w8_i1": w8_i1, "wb_i1": wb_i1,
            "w8_d2": w8_d2, "wb_d2": wb_d2,
            "wrow_b": wrow_b, "wrow_d": wrow_d,
        })
    return in_maps


def kernel(**inputs):
    from concourse.bass_utils import run_bass_kernel_spmd

    nc = _get_nc()
    in_maps = make_in_maps(inputs)
    res = run_bass_kernel_spmd(nc, in_maps, list(range(NCORES))).results
    rows = np.concatenate([res[k]["out_rows"] for k in range(NCORES)], axis=0)
    att_img = np.ascontiguousarray(
        np.broadcast_to(rows[:, 0][:, None, :], (B, S, H)))
    att_dns = np.ascontiguousarray(
        np.broadcast_to(rows[:, 1][:, None, :], (B, S, H)))
    return att_dns, att_img
